# revision 8
# baseline (speedup 1.0000x reference)
"""DeepSeek-style MoE layer on 8 Trainium2 NeuronCores, expert-parallel.

Strategy:
  - Routing (sigmoid gate + group-limited top-k) and token dispatch/combine
    indices are computed on host in fp32 numpy (replicating the reference
    semantics exactly, ~0.1% of total FLOPs).
  - The 32 experts are bin-packed onto 8 cores (4 slots/core) by measured
    load, with per-slot static capacities padded to multiples of 128.
  - Each core runs a Bass/Tile kernel: per slot s,
        gT = silu(w1_s^T @ bufT_s)        [I, Cs]
        hT = gT * (w3_s^T @ bufT_s)       [I, Cs]
        y  = hT^T-form @ w2_s             [Cs, H]
    All matmuls in bf16 with fp32 PSUM accumulation.
  - Combine (gather + weighted sum over the K=8 routes) happens on host.
"""

import math

import ml_dtypes
import numpy as np

import concourse.bass as bass
import concourse.mybir as mybir
import concourse.tile as tile
from concourse import bacc
from concourse.bass_utils import run_bass_kernel_spmd
from concourse.kernels.tile_matmul import (
    composable_matmul_tile_kernel,
    dma_from_dram_kxm,
    dma_from_dram_kxn,
    dma_from_dram_mxn,
    dma_to_dram_mxn,
    k_pool_min_bufs,
    scalar_copyback,
)

# MoE config (matches the reference)
N = 2048
H = 2048
I = 1024
E = 32
K = 8
G = 8
KG = 4
C = 1024
SCALE = 2.5

M_CORES = 8
SLOTS = E // M_CORES  # 4 experts per core

BF16 = ml_dtypes.bfloat16


def _route(x, w_gate, gate_bias):
    """fp32 numpy replication of the reference gate."""
    scores = 1.0 / (1.0 + np.exp(-(x @ w_gate), dtype=np.float32))  # [N, E]
    sb = scores + gate_bias
    grp = sb.reshape(N, G, E // G)
    top2 = -np.sort(-grp, axis=-1)[..., :2]
    gscore = top2.sum(-1)  # [N, G]
    gidx = np.argsort(-gscore, axis=-1, kind="stable")[:, :KG]
    gmask = np.zeros((N, G), bool)
    gmask[np.arange(N)[:, None], gidx] = True
    emask = np.repeat(gmask, E // G, axis=1)
    masked = np.where(emask, sb, -np.inf)
    eidx = np.argsort(-masked, axis=-1, kind="stable")[:, :K]  # [N, K]
    w = np.take_along_axis(scores, eidx, axis=1)
    w = w / w.sum(-1, keepdims=True) * SCALE
    return eidx, w.astype(np.float32)


def _dispatch_indices(eidx):
    """Per-route slot positions, replicating the reference capacity rule."""
    flat_e = eidx.reshape(-1)  # [N*K], token-major arrival order
    tok = np.repeat(np.arange(N), K)
    order = np.argsort(flat_e, kind="stable")
    counts = np.bincount(flat_e, minlength=E)
    starts = np.concatenate([[0], np.cumsum(counts)[:-1]])
    pos_sorted = np.arange(N * K) - np.repeat(starts, counts)
    pos = np.empty(N * K, np.int64)
    pos[order] = pos_sorted
    valid = pos < C
    return flat_e, tok, pos, valid, counts


def _assign_experts(counts):
    """Bin-pack experts: slot s takes load-ranks [8s, 8s+8), snaked across cores."""
    loads = np.minimum(counts, C)
    rank = np.argsort(-loads, kind="stable")
    expert_core = np.empty(E, np.int64)
    expert_slot = np.empty(E, np.int64)
    caps = []
    for s in range(SLOTS):
        grp = rank[M_CORES * s : M_CORES * (s + 1)]
        cores = range(M_CORES) if s % 2 == 0 else range(M_CORES - 1, -1, -1)
        for c, e in zip(cores, grp):
            expert_core[e] = c
            expert_slot[e] = s
        caps.append(int(math.ceil(max(int(loads[e]) for e in grp) / 128.0) * 128))
    offs = np.concatenate([[0], np.cumsum(caps)]).astype(np.int64)
    return expert_core, expert_slot, caps, offs


def _memo_producer(producer):
    memo = {}

    def _p(nc, md):
        key = (md.k_batch_idx, md.k_tile_idx, md.n_batch_idx, md.n_tile_idx)
        if key not in memo:
            memo[key] = producer(nc, md)
        return memo[key]

    return _p


def _slot_graph(ctx, tc, s, Cs, w1_ap, w3_ap, w2_ap, buf_ap, y_ap):
    """One expert slot: gT = silu(w1^T buf), hT = gT * (w3^T buf), y = hT^T-form w2.
    gT/hT stay resident in SBUF; buf is DMA'd once and shared by both mm1s."""
    nc = tc.nc
    P = 128
    ISUB = I // P  # 8
    Cp = int(math.ceil(Cs / 512.0) * 512)  # pad free dim so n-tile views stay in range

    spool = ctx.enter_context(tc.tile_pool(name=f"sb{s}", bufs=1))
    gt_sb = spool.tile([P, ISUB, Cp], mybir.dt.bfloat16, name=f"gt{s}")
    ht_sb = spool.tile([P, ISUB, Cp], mybir.dt.bfloat16, name=f"ht{s}")

    # whole [H, Cs] token block resident in SBUF, loaded once
    HSUB = H // P  # 16
    buf_sb = spool.tile([P, HSUB, Cp], mybir.dt.bfloat16, name=f"bs{s}")
    buf_3d = buf_ap.rearrange("(po pi) f -> pi po f", pi=P)
    nc.sync.dma_start(out=buf_sb[:, :, :Cs], in_=buf_3d)

    from concourse.kernels.tile_matmul import ShapeInfo

    buf_shape = ShapeInfo(pdims=((P, HSUB),), fdims=(Cs,))

    def buf_producer(nc_, md):
        return buf_sb[:, bass.ds(md.k_tile_idx * md.k_subtiles, md.k_subtiles),
                      bass.ds(md.n_tile_idx * md.n_tile, md.n_tile)]

    w1_pool = ctx.enter_context(tc.tile_pool(name=f"w1p{s}", bufs=5))
    w3_pool = ctx.enter_context(tc.tile_pool(name=f"w3p{s}", bufs=5))
    w1_producer, w1_shape = dma_from_dram_kxm(w1_pool, w1_ap)
    w3_producer, _ = dma_from_dram_kxm(w3_pool, w3_ap)

    def gt_view(md):
        return gt_sb[:, bass.ts(md.m_tile_idx, md.m_subtiles),
                     bass.ds(md.n_tile_idx * md.n_tile, md.n_tile)]

    def ht_view(md):
        return ht_sb[:, bass.ts(md.m_tile_idx, md.m_subtiles),
                     bass.ds(md.n_tile_idx * md.n_tile, md.n_tile)]

    def silu_reduce(nc_, psum, sbuf, md):
        nc_.scalar.activation(sbuf[:], psum[:], mybir.ActivationFunctionType.Silu)

    def mul_reduce(nc_, psum, sbuf, md):
        g = gt_sb[:, bass.ds(md.m_tile_idx * md.m_subtiles + md.m_subtile_idx, 1),
                  bass.ds(md.n_tile_idx * md.n_tile + md.n_subtile_idx * md.n_subtile,
                          md.n_subtile)]
        nc_.vector.tensor_mul(out=sbuf[:], in0=psum[:], in1=g)

    def noop_consumer(nc_, sbuf, md):
        pass

    # mm1a: gT = silu(w1^T @ buf)
    composable_matmul_tile_kernel(
        tc=tc, kxm_shape=w1_shape, kxn_shape=buf_shape, output_type=mybir.dt.bfloat16,
        kxm_producer=w1_producer, kxn_producer=buf_producer,
        mxn_consumer=noop_consumer, mxn_subtile_reducer=silu_reduce,
        mxn_subtile_producer=lambda nc_, md: gt_view(md), psum_n_bufs=2,
    )
    # mm1b: hT = gT * (w3^T @ buf)
    composable_matmul_tile_kernel(
        tc=tc, kxm_shape=w1_shape, kxn_shape=buf_shape, output_type=mybir.dt.bfloat16,
        kxm_producer=w3_producer, kxn_producer=buf_producer,
        mxn_consumer=noop_consumer, mxn_subtile_reducer=mul_reduce,
        mxn_subtile_producer=lambda nc_, md: ht_view(md), psum_n_bufs=2,
    )

    # mm2: y = hT^T-form @ w2 ; kxm tiles are views of ht_sb
    ht_shape = ShapeInfo(pdims=((P, ISUB),), fdims=(Cs,))

    def ht_kxm_producer(nc_, md):
        return ht_sb[:, bass.ds(md.k_tile_idx * md.k_subtiles, md.k_subtiles),
                     bass.ds(md.m_tile_idx * md.m_tile, md.m_tile)]

    w2_pool = ctx.enter_context(tc.tile_pool(name=f"w2p{s}", bufs=3))
    w2_producer, w2_shape = dma_from_dram_kxn(w2_pool, w2_ap)
    composable_matmul_tile_kernel(
        tc=tc, kxm_shape=ht_shape, kxn_shape=w2_shape, output_type=mybir.dt.float32,
        kxm_producer=ht_kxm_producer, kxn_producer=w2_producer,
        mxn_consumer=dma_to_dram_mxn(y_ap),
        mxn_subtile_reducer=scalar_copyback(), psum_n_bufs=2,
    )


def _build_graph(caps, offs):
    from contextlib import ExitStack

    CT = int(offs[-1])
    nc = bacc.Bacc(None, target_bir_lowering=False, debug=False)
    w1_p = nc.declare_dram_parameter("w1s", [SLOTS, H, I], mybir.dt.bfloat16, isOutput=False)
    w3_p = nc.declare_dram_parameter("w3s", [SLOTS, H, I], mybir.dt.bfloat16, isOutput=False)
    w2_p = nc.declare_dram_parameter("w2s", [SLOTS, I, H], mybir.dt.bfloat16, isOutput=False)
    buf_p = nc.declare_dram_parameter("buft", [H, CT], mybir.dt.bfloat16, isOutput=False)
    y_p = nc.declare_dram_parameter("y", [CT, H], mybir.dt.float32, isOutput=True)

    with tile.TileContext(nc) as tc:
        for s in range(SLOTS):
            sl = bass.ds(int(offs[s]), caps[s])
            with ExitStack() as sctx:
                _slot_graph(sctx, tc, s, caps[s], w1_p[s], w3_p[s], w2_p[s],
                            buf_p[:, sl], y_p[sl, :])
    nc.compile()
    return nc


_GRAPH_CACHE = {}


def kernel(x, w_gate, gate_bias, w1, w3, w2):
    x = np.asarray(x, np.float32)
    w_gate = np.asarray(w_gate, np.float32)
    gate_bias = np.asarray(gate_bias, np.float32)

    eidx, w = _route(x, w_gate, gate_bias)
    flat_e, tok, pos, valid, counts = _dispatch_indices(eidx)
    expert_core, expert_slot, caps, offs = _assign_experts(counts)
    CT = int(offs[-1])

    # ---- stage per-core inputs ----
    bufT = np.zeros((M_CORES, H, CT), BF16)
    expert_tokens = {}
    for e in range(E):
        m = (flat_e == e) & valid
        toks = tok[m]  # arrival order == slot positions 0..len-1
        expert_tokens[e] = toks
        c, s = int(expert_core[e]), int(expert_slot[e])
        o = int(offs[s])
        bufT[c][:, o : o + len(toks)] = x[toks].astype(BF16).T

    w1b = np.asarray(w1, np.float32).astype(BF16)
    w3b = np.asarray(w3, np.float32).astype(BF16)
    w2b = np.asarray(w2, np.float32).astype(BF16)
    in_maps = []
    for c in range(M_CORES):
        # experts of core c in slot order
        es = [int(np.where((expert_core == c) & (expert_slot == s))[0][0]) for s in range(SLOTS)]
        in_maps.append({
            "w1s": np.ascontiguousarray(w1b[es]),
            "w3s": np.ascontiguousarray(w3b[es]),
            "w2s": np.ascontiguousarray(w2b[es]),
            "buft": bufT[c],
        })

    # ---- build + run ----
    key = tuple(caps)
    if key not in _GRAPH_CACHE:
        _GRAPH_CACHE[key] = _build_graph(caps, offs)
    nc = _GRAPH_CACHE[key]
    res = run_bass_kernel_spmd(nc, in_maps, core_ids=list(range(M_CORES)))
    ys = np.stack([res.results[c]["y"] for c in range(M_CORES)])  # [8, CT, H] f32

    # ---- combine on host ----
    ycat = ys.reshape(M_CORES * CT, H)
    core_of = expert_core[flat_e]
    slot_of = expert_slot[flat_e]
    col = offs[slot_of] + pos
    row = core_of * CT + col
    row = np.where(valid, row, 0)
    wf = np.where(valid, w.reshape(-1), 0.0).astype(np.float32)
    contrib = ycat[row] * wf[:, None]  # [N*K, H]
    out = contrib.reshape(N, K, H).sum(axis=1)
    return out.astype(np.float32)


# revision 9
# speedup vs baseline: 1.0361x; 1.0361x over previous
"""DeepSeek-style MoE layer on 8 Trainium2 NeuronCores, expert-parallel.

Strategy:
  - Routing (sigmoid gate + group-limited top-k) and token dispatch/combine
    indices are computed on host in fp32 numpy (replicating the reference
    semantics exactly, ~0.1% of total FLOPs).
  - The 32 experts are bin-packed onto 8 cores (4 slots/core) by measured
    load, with per-slot static capacities padded to multiples of 128.
  - Each core runs a Bass/Tile kernel: per slot s,
        gT = silu(w1_s^T @ bufT_s)        [I, Cs]
        hT = gT * (w3_s^T @ bufT_s)       [I, Cs]
        y  = hT^T-form @ w2_s             [Cs, H]
    All matmuls in bf16 with fp32 PSUM accumulation.
  - Combine (gather + weighted sum over the K=8 routes) happens on host.
"""

import math

import ml_dtypes
import numpy as np

import concourse.bass as bass
import concourse.mybir as mybir
import concourse.tile as tile
from concourse import bacc
from concourse.bass_utils import run_bass_kernel_spmd
from concourse.kernels.tile_matmul import (
    composable_matmul_tile_kernel,
    dma_from_dram_kxm,
    dma_from_dram_kxn,
    dma_from_dram_mxn,
    dma_to_dram_mxn,
    k_pool_min_bufs,
    scalar_copyback,
)

# MoE config (matches the reference)
N = 2048
H = 2048
I = 1024
E = 32
K = 8
G = 8
KG = 4
C = 1024
SCALE = 2.5

M_CORES = 8
SLOTS = E // M_CORES  # 4 experts per core

BF16 = ml_dtypes.bfloat16


def _route(x, w_gate, gate_bias):
    """fp32 numpy replication of the reference gate."""
    scores = 1.0 / (1.0 + np.exp(-(x @ w_gate), dtype=np.float32))  # [N, E]
    sb = scores + gate_bias
    grp = sb.reshape(N, G, E // G)
    top2 = -np.sort(-grp, axis=-1)[..., :2]
    gscore = top2.sum(-1)  # [N, G]
    gidx = np.argsort(-gscore, axis=-1, kind="stable")[:, :KG]
    gmask = np.zeros((N, G), bool)
    gmask[np.arange(N)[:, None], gidx] = True
    emask = np.repeat(gmask, E // G, axis=1)
    masked = np.where(emask, sb, -np.inf)
    eidx = np.argsort(-masked, axis=-1, kind="stable")[:, :K]  # [N, K]
    w = np.take_along_axis(scores, eidx, axis=1)
    w = w / w.sum(-1, keepdims=True) * SCALE
    return eidx, w.astype(np.float32)


def _dispatch_indices(eidx):
    """Per-route slot positions, replicating the reference capacity rule."""
    flat_e = eidx.reshape(-1)  # [N*K], token-major arrival order
    tok = np.repeat(np.arange(N), K)
    order = np.argsort(flat_e, kind="stable")
    counts = np.bincount(flat_e, minlength=E)
    starts = np.concatenate([[0], np.cumsum(counts)[:-1]])
    pos_sorted = np.arange(N * K) - np.repeat(starts, counts)
    pos = np.empty(N * K, np.int64)
    pos[order] = pos_sorted
    valid = pos < C
    return flat_e, tok, pos, valid, counts


def _assign_experts(counts):
    """Bin-pack experts: slot s takes load-ranks [8s, 8s+8), snaked across cores."""
    loads = np.minimum(counts, C)
    rank = np.argsort(-loads, kind="stable")
    expert_core = np.empty(E, np.int64)
    expert_slot = np.empty(E, np.int64)
    caps = []
    for s in range(SLOTS):
        grp = rank[M_CORES * s : M_CORES * (s + 1)]
        cores = range(M_CORES) if s % 2 == 0 else range(M_CORES - 1, -1, -1)
        for c, e in zip(cores, grp):
            expert_core[e] = c
            expert_slot[e] = s
        caps.append(int(math.ceil(max(int(loads[e]) for e in grp) / 128.0) * 128))
    offs = np.concatenate([[0], np.cumsum(caps)]).astype(np.int64)
    return expert_core, expert_slot, caps, offs


def _memo_producer(producer):
    memo = {}

    def _p(nc, md):
        key = (md.k_batch_idx, md.k_tile_idx, md.n_batch_idx, md.n_tile_idx)
        if key not in memo:
            memo[key] = producer(nc, md)
        return memo[key]

    return _p


def _slot_graph(ctx, tc, s, Cs, w1_ap, w3_ap, w2_ap, buf_ap, y_ap):
    """One expert slot: gT = silu(w1^T buf), hT = gT * (w3^T buf), y = hT^T-form w2.
    gT/hT stay resident in SBUF; buf is DMA'd once and shared by both mm1s."""
    nc = tc.nc
    P = 128
    ISUB = I // P  # 8
    Cp = int(math.ceil(Cs / 512.0) * 512)  # pad free dim so n-tile views stay in range

    spool = ctx.enter_context(tc.tile_pool(name=f"sb{s}", bufs=1))
    gt_sb = spool.tile([P, ISUB, Cp], mybir.dt.bfloat16, name=f"gt{s}")
    ht_sb = spool.tile([P, ISUB, Cp], mybir.dt.bfloat16, name=f"ht{s}")

    from concourse.kernels.tile_matmul import ShapeInfo

    n_buf_tiles = int(math.ceil(Cs / 512.0)) * (H // 512)  # live buf tiles per slot
    bpool = ctx.enter_context(tc.tile_pool(name=f"bf{s}", bufs=n_buf_tiles + 1))
    buf_producer_raw, buf_shape = dma_from_dram_kxn(bpool, buf_ap)
    buf_producer = _memo_producer(buf_producer_raw)

    w1_pool = ctx.enter_context(tc.tile_pool(name=f"w1p{s}", bufs=5))
    w3_pool = ctx.enter_context(tc.tile_pool(name=f"w3p{s}", bufs=5))
    w1_producer, w1_shape = dma_from_dram_kxm(w1_pool, w1_ap)
    w3_producer, _ = dma_from_dram_kxm(w3_pool, w3_ap)

    def gt_view(md):
        return gt_sb[:, bass.ts(md.m_tile_idx, md.m_subtiles),
                     bass.ds(md.n_tile_idx * md.n_tile, md.n_tile)]

    def ht_view(md):
        return ht_sb[:, bass.ts(md.m_tile_idx, md.m_subtiles),
                     bass.ds(md.n_tile_idx * md.n_tile, md.n_tile)]

    def silu_reduce(nc_, psum, sbuf, md):
        nc_.scalar.activation(sbuf[:], psum[:], mybir.ActivationFunctionType.Silu)

    def mul_reduce(nc_, psum, sbuf, md):
        g = gt_sb[:, bass.ds(md.m_tile_idx * md.m_subtiles + md.m_subtile_idx, 1),
                  bass.ds(md.n_tile_idx * md.n_tile + md.n_subtile_idx * md.n_subtile,
                          md.n_subtile)]
        nc_.vector.tensor_mul(out=sbuf[:], in0=psum[:], in1=g)

    def noop_consumer(nc_, sbuf, md):
        pass

    # mm1a: gT = silu(w1^T @ buf)
    composable_matmul_tile_kernel(
        tc=tc, kxm_shape=w1_shape, kxn_shape=buf_shape, output_type=mybir.dt.bfloat16,
        kxm_producer=w1_producer, kxn_producer=buf_producer,
        mxn_consumer=noop_consumer, mxn_subtile_reducer=silu_reduce,
        mxn_subtile_producer=lambda nc_, md: gt_view(md), psum_n_bufs=1,
    )
    # mm1b: hT = gT * (w3^T @ buf)
    composable_matmul_tile_kernel(
        tc=tc, kxm_shape=w1_shape, kxn_shape=buf_shape, output_type=mybir.dt.bfloat16,
        kxm_producer=w3_producer, kxn_producer=buf_producer,
        mxn_consumer=noop_consumer, mxn_subtile_reducer=mul_reduce,
        mxn_subtile_producer=lambda nc_, md: ht_view(md), psum_n_bufs=1,
    )

    # mm2: y = hT^T-form @ w2 ; kxm tiles are views of ht_sb
    ht_shape = ShapeInfo(pdims=((P, ISUB),), fdims=(Cs,))

    def ht_kxm_producer(nc_, md):
        return ht_sb[:, bass.ds(md.k_tile_idx * md.k_subtiles, md.k_subtiles),
                     bass.ds(md.m_tile_idx * md.m_tile, md.m_tile)]

    w2_pool = ctx.enter_context(tc.tile_pool(name=f"w2p{s}", bufs=3))
    w2_producer, w2_shape = dma_from_dram_kxn(w2_pool, w2_ap)
    composable_matmul_tile_kernel(
        tc=tc, kxm_shape=ht_shape, kxn_shape=w2_shape, output_type=mybir.dt.float32,
        kxm_producer=ht_kxm_producer, kxn_producer=w2_producer,
        mxn_consumer=dma_to_dram_mxn(y_ap),
        mxn_subtile_reducer=scalar_copyback(), psum_n_bufs=1,
    )


def _build_graph(caps, offs):
    from contextlib import ExitStack

    CT = int(offs[-1])
    nc = bacc.Bacc(None, target_bir_lowering=False, debug=False)
    w1_p = nc.declare_dram_parameter("w1s", [SLOTS, H, I], mybir.dt.bfloat16, isOutput=False)
    w3_p = nc.declare_dram_parameter("w3s", [SLOTS, H, I], mybir.dt.bfloat16, isOutput=False)
    w2_p = nc.declare_dram_parameter("w2s", [SLOTS, I, H], mybir.dt.bfloat16, isOutput=False)
    buf_p = nc.declare_dram_parameter("buft", [H, CT], mybir.dt.bfloat16, isOutput=False)
    y_p = nc.declare_dram_parameter("y", [CT, H], mybir.dt.float32, isOutput=True)

    with tile.TileContext(nc) as tc:
        for s in range(SLOTS):
            sl = bass.ds(int(offs[s]), caps[s])
            with ExitStack() as sctx:
                _slot_graph(sctx, tc, s, caps[s], w1_p[s], w3_p[s], w2_p[s],
                            buf_p[:, sl], y_p[sl, :])
    nc.compile()
    return nc


_GRAPH_CACHE = {}


def kernel(x, w_gate, gate_bias, w1, w3, w2):
    x = np.asarray(x, np.float32)
    w_gate = np.asarray(w_gate, np.float32)
    gate_bias = np.asarray(gate_bias, np.float32)

    eidx, w = _route(x, w_gate, gate_bias)
    flat_e, tok, pos, valid, counts = _dispatch_indices(eidx)
    expert_core, expert_slot, caps, offs = _assign_experts(counts)
    CT = int(offs[-1])

    # ---- stage per-core inputs ----
    bufT = np.zeros((M_CORES, H, CT), BF16)
    expert_tokens = {}
    for e in range(E):
        m = (flat_e == e) & valid
        toks = tok[m]  # arrival order == slot positions 0..len-1
        expert_tokens[e] = toks
        c, s = int(expert_core[e]), int(expert_slot[e])
        o = int(offs[s])
        bufT[c][:, o : o + len(toks)] = x[toks].astype(BF16).T

    w1b = np.asarray(w1, np.float32).astype(BF16)
    w3b = np.asarray(w3, np.float32).astype(BF16)
    w2b = np.asarray(w2, np.float32).astype(BF16)
    in_maps = []
    for c in range(M_CORES):
        # experts of core c in slot order
        es = [int(np.where((expert_core == c) & (expert_slot == s))[0][0]) for s in range(SLOTS)]
        in_maps.append({
            "w1s": np.ascontiguousarray(w1b[es]),
            "w3s": np.ascontiguousarray(w3b[es]),
            "w2s": np.ascontiguousarray(w2b[es]),
            "buft": bufT[c],
        })

    # ---- build + run ----
    key = tuple(caps)
    if key not in _GRAPH_CACHE:
        _GRAPH_CACHE[key] = _build_graph(caps, offs)
    nc = _GRAPH_CACHE[key]
    res = run_bass_kernel_spmd(nc, in_maps, core_ids=list(range(M_CORES)))
    ys = np.stack([res.results[c]["y"] for c in range(M_CORES)])  # [8, CT, H] f32

    # ---- combine on host ----
    ycat = ys.reshape(M_CORES * CT, H)
    core_of = expert_core[flat_e]
    slot_of = expert_slot[flat_e]
    col = offs[slot_of] + pos
    row = core_of * CT + col
    row = np.where(valid, row, 0)
    wf = np.where(valid, w.reshape(-1), 0.0).astype(np.float32)
    contrib = ycat[row] * wf[:, None]  # [N*K, H]
    out = contrib.reshape(N, K, H).sum(axis=1)
    return out.astype(np.float32)


# revision 13
# speedup vs baseline: 1.1431x; 1.1033x over previous
"""DeepSeek-style MoE layer on 8 Trainium2 NeuronCores, expert-parallel.

Strategy:
  - Routing (sigmoid gate + group-limited top-k) and token dispatch/combine
    indices are computed on host in fp32 numpy (replicating the reference
    semantics exactly, ~0.1% of total FLOPs).
  - The 32 experts are bin-packed onto 8 cores (4 slots/core) by measured
    load, with per-slot static capacities padded to multiples of 128.
  - Each core runs a Bass/Tile kernel: per slot s,
        gT = silu(w1_s^T @ bufT_s)        [I, Cs]
        hT = gT * (w3_s^T @ bufT_s)       [I, Cs]
        y  = hT^T-form @ w2_s             [Cs, H]
    All matmuls in bf16 with fp32 PSUM accumulation.
  - Combine (gather + weighted sum over the K=8 routes) happens on host.
"""

import math

import ml_dtypes
import numpy as np

import concourse.bass as bass
import concourse.mybir as mybir
import concourse.tile as tile
from concourse import bacc
from concourse.bass_utils import run_bass_kernel_spmd
from concourse.kernels.tile_matmul import (
    composable_matmul_tile_kernel,
    dma_from_dram_kxm,
    dma_from_dram_kxn,
    dma_from_dram_mxn,
    dma_to_dram_mxn,
    k_pool_min_bufs,
    scalar_copyback,
)

# MoE config (matches the reference)
N = 2048
H = 2048
I = 1024
E = 32
K = 8
G = 8
KG = 4
C = 1024
SCALE = 2.5

M_CORES = 8
SLOTS = E // M_CORES  # 4 experts per core

BF16 = ml_dtypes.bfloat16


def _route(x, w_gate, gate_bias):
    """fp32 numpy replication of the reference gate."""
    scores = 1.0 / (1.0 + np.exp(-(x @ w_gate), dtype=np.float32))  # [N, E]
    sb = scores + gate_bias
    grp = sb.reshape(N, G, E // G)
    top2 = -np.sort(-grp, axis=-1)[..., :2]
    gscore = top2.sum(-1)  # [N, G]
    gidx = np.argsort(-gscore, axis=-1, kind="stable")[:, :KG]
    gmask = np.zeros((N, G), bool)
    gmask[np.arange(N)[:, None], gidx] = True
    emask = np.repeat(gmask, E // G, axis=1)
    masked = np.where(emask, sb, -np.inf)
    eidx = np.argsort(-masked, axis=-1, kind="stable")[:, :K]  # [N, K]
    w = np.take_along_axis(scores, eidx, axis=1)
    w = w / w.sum(-1, keepdims=True) * SCALE
    return eidx, w.astype(np.float32)


def _dispatch_indices(eidx):
    """Per-route slot positions, replicating the reference capacity rule."""
    flat_e = eidx.reshape(-1)  # [N*K], token-major arrival order
    tok = np.repeat(np.arange(N), K)
    order = np.argsort(flat_e, kind="stable")
    counts = np.bincount(flat_e, minlength=E)
    starts = np.concatenate([[0], np.cumsum(counts)[:-1]])
    pos_sorted = np.arange(N * K) - np.repeat(starts, counts)
    pos = np.empty(N * K, np.int64)
    pos[order] = pos_sorted
    valid = pos < C
    return flat_e, tok, pos, valid, counts


def _assign_experts(counts):
    """Bin-pack experts: slot s takes load-ranks [8s, 8s+8), snaked across cores."""
    loads = np.minimum(counts, C)
    rank = np.argsort(-loads, kind="stable")
    expert_core = np.empty(E, np.int64)
    expert_slot = np.empty(E, np.int64)
    caps = []
    for s in range(SLOTS):
        grp = rank[M_CORES * s : M_CORES * (s + 1)]
        cores = range(M_CORES) if s % 2 == 0 else range(M_CORES - 1, -1, -1)
        for c, e in zip(cores, grp):
            expert_core[e] = c
            expert_slot[e] = s
        caps.append(int(math.ceil(max(int(loads[e]) for e in grp) / 128.0) * 128))
    offs = np.concatenate([[0], np.cumsum(caps)]).astype(np.int64)
    return expert_core, expert_slot, caps, offs


def _memo_producer(producer):
    memo = {}

    def _p(nc, md):
        key = (md.k_batch_idx, md.k_tile_idx, md.n_batch_idx, md.n_tile_idx)
        if key not in memo:
            memo[key] = producer(nc, md)
        return memo[key]

    return _p


def _slot_graph(ctx, tc, s, Cs, w1_ap, w3_ap, w2_ap, buf_ap, y_ap):
    """One expert slot: gT = silu(w1^T buf), hT = gT * (w3^T buf), y = hT^T-form w2.
    gT/hT stay resident in SBUF; buf and w2 are DMA'd once (memoized producers)."""
    nc = tc.nc
    P = 128
    ISUB = I // P  # 8
    Cp = int(math.ceil(Cs / 512.0) * 512)  # pad free dim so n-tile views stay in range

    spool = ctx.enter_context(tc.tile_pool(name=f"sb{s}", bufs=1))
    gt_sb = spool.tile([P, ISUB, Cp], mybir.dt.bfloat16, name=f"gt{s}")
    ht_sb = spool.tile([P, ISUB, Cp], mybir.dt.bfloat16, name=f"ht{s}")

    from concourse.kernels.tile_matmul import ShapeInfo

    n_buf_tiles = int(math.ceil(Cs / 512.0)) * (H // 512)  # live buf tiles per slot
    bpool = ctx.enter_context(tc.tile_pool(name=f"bf{s}", bufs=n_buf_tiles + 1))
    buf_producer_raw, buf_shape = dma_from_dram_kxn(bpool, buf_ap)
    buf_producer = _memo_producer(buf_producer_raw)

    w1_pool = ctx.enter_context(tc.tile_pool(name=f"w1p{s}", bufs=6))
    w3_pool = ctx.enter_context(tc.tile_pool(name=f"w3p{s}", bufs=6))
    w1_producer, w1_shape = dma_from_dram_kxm(w1_pool, w1_ap)
    w3_producer, _ = dma_from_dram_kxm(w3_pool, w3_ap)

    def gt_view(md):
        return gt_sb[:, bass.ts(md.m_tile_idx, md.m_subtiles),
                     bass.ds(md.n_tile_idx * md.n_tile, md.n_tile)]

    def ht_view(md):
        return ht_sb[:, bass.ts(md.m_tile_idx, md.m_subtiles),
                     bass.ds(md.n_tile_idx * md.n_tile, md.n_tile)]

    def silu_reduce(nc_, psum, sbuf, md):
        nc_.scalar.activation(sbuf[:], psum[:], mybir.ActivationFunctionType.Silu)

    def mul_reduce(nc_, psum, sbuf, md):
        g = gt_sb[:, bass.ds(md.m_tile_idx * md.m_subtiles + md.m_subtile_idx, 1),
                  bass.ds(md.n_tile_idx * md.n_tile + md.n_subtile_idx * md.n_subtile,
                          md.n_subtile)]
        nc_.vector.tensor_mul(out=sbuf[:], in0=psum[:], in1=g)

    def noop_consumer(nc_, sbuf, md):
        pass

    # mm1a: gT = silu(w1^T @ buf)
    composable_matmul_tile_kernel(
        tc=tc, kxm_shape=w1_shape, kxn_shape=buf_shape, output_type=mybir.dt.bfloat16,
        kxm_producer=w1_producer, kxn_producer=buf_producer,
        mxn_consumer=noop_consumer, mxn_subtile_reducer=silu_reduce,
        mxn_subtile_producer=lambda nc_, md: gt_view(md), psum_n_bufs=1,
    )
    # mm1b: hT = gT * (w3^T @ buf)
    composable_matmul_tile_kernel(
        tc=tc, kxm_shape=w1_shape, kxn_shape=buf_shape, output_type=mybir.dt.bfloat16,
        kxm_producer=w3_producer, kxn_producer=buf_producer,
        mxn_consumer=noop_consumer, mxn_subtile_reducer=mul_reduce,
        mxn_subtile_producer=lambda nc_, md: ht_view(md), psum_n_bufs=1,
    )

    # mm2: y = hT^T-form @ w2 ; kxm tiles are views of ht_sb
    ht_shape = ShapeInfo(pdims=((P, ISUB),), fdims=(Cs,))

    def ht_kxm_producer(nc_, md):
        return ht_sb[:, bass.ds(md.k_tile_idx * md.k_subtiles, md.k_subtiles),
                     bass.ds(md.m_tile_idx * md.m_tile, md.m_tile)]

    w2_pool = ctx.enter_context(tc.tile_pool(name=f"w2p{s}", bufs=9))
    w2_producer_raw, w2_shape = dma_from_dram_kxn(w2_pool, w2_ap)
    w2_producer = _memo_producer(w2_producer_raw)
    composable_matmul_tile_kernel(
        tc=tc, kxm_shape=ht_shape, kxn_shape=w2_shape, output_type=mybir.dt.float32,
        kxm_producer=ht_kxm_producer, kxn_producer=w2_producer,
        mxn_consumer=dma_to_dram_mxn(y_ap),
        mxn_subtile_reducer=scalar_copyback(), psum_n_bufs=1,
    )


def _build_graph(caps, offs):
    from contextlib import ExitStack

    CT = int(offs[-1])
    nc = bacc.Bacc(None, target_bir_lowering=False, debug=False)
    w1_p = nc.declare_dram_parameter("w1s", [SLOTS, H, I], mybir.dt.bfloat16, isOutput=False)
    w3_p = nc.declare_dram_parameter("w3s", [SLOTS, H, I], mybir.dt.bfloat16, isOutput=False)
    w2_p = nc.declare_dram_parameter("w2s", [SLOTS, I, H], mybir.dt.bfloat16, isOutput=False)
    buf_p = nc.declare_dram_parameter("buft", [H, CT], mybir.dt.bfloat16, isOutput=False)
    y_p = nc.declare_dram_parameter("y", [CT, H], mybir.dt.float32, isOutput=True)

    with tile.TileContext(nc) as tc:
        for s in range(SLOTS):
            sl = bass.ds(int(offs[s]), caps[s])
            with ExitStack() as sctx:
                _slot_graph(sctx, tc, s, caps[s], w1_p[s], w3_p[s], w2_p[s],
                            buf_p[:, sl], y_p[sl, :])
    nc.compile()
    return nc


_GRAPH_CACHE = {}


def kernel(x, w_gate, gate_bias, w1, w3, w2):
    x = np.asarray(x, np.float32)
    w_gate = np.asarray(w_gate, np.float32)
    gate_bias = np.asarray(gate_bias, np.float32)

    eidx, w = _route(x, w_gate, gate_bias)
    flat_e, tok, pos, valid, counts = _dispatch_indices(eidx)
    expert_core, expert_slot, caps, offs = _assign_experts(counts)
    CT = int(offs[-1])

    # ---- stage per-core inputs ----
    bufT = np.zeros((M_CORES, H, CT), BF16)
    expert_tokens = {}
    for e in range(E):
        m = (flat_e == e) & valid
        toks = tok[m]  # arrival order == slot positions 0..len-1
        expert_tokens[e] = toks
        c, s = int(expert_core[e]), int(expert_slot[e])
        o = int(offs[s])
        bufT[c][:, o : o + len(toks)] = x[toks].astype(BF16).T

    w1b = np.asarray(w1, np.float32).astype(BF16)
    w3b = np.asarray(w3, np.float32).astype(BF16)
    w2b = np.asarray(w2, np.float32).astype(BF16)
    in_maps = []
    for c in range(M_CORES):
        # experts of core c in slot order
        es = [int(np.where((expert_core == c) & (expert_slot == s))[0][0]) for s in range(SLOTS)]
        in_maps.append({
            "w1s": np.ascontiguousarray(w1b[es]),
            "w3s": np.ascontiguousarray(w3b[es]),
            "w2s": np.ascontiguousarray(w2b[es]),
            "buft": bufT[c],
        })

    # ---- build + run ----
    key = tuple(caps)
    if key not in _GRAPH_CACHE:
        _GRAPH_CACHE[key] = _build_graph(caps, offs)
    nc = _GRAPH_CACHE[key]
    res = run_bass_kernel_spmd(nc, in_maps, core_ids=list(range(M_CORES)))
    ys = np.stack([res.results[c]["y"] for c in range(M_CORES)])  # [8, CT, H] f32

    # ---- combine on host ----
    ycat = ys.reshape(M_CORES * CT, H)
    core_of = expert_core[flat_e]
    slot_of = expert_slot[flat_e]
    col = offs[slot_of] + pos
    row = core_of * CT + col
    row = np.where(valid, row, 0)
    wf = np.where(valid, w.reshape(-1), 0.0).astype(np.float32)
    contrib = ycat[row] * wf[:, None]  # [N*K, H]
    out = contrib.reshape(N, K, H).sum(axis=1)
    return out.astype(np.float32)


# revision 14
# speedup vs baseline: 1.1588x; 1.0137x over previous
"""DeepSeek-style MoE layer on 8 Trainium2 NeuronCores, expert-parallel.

Strategy:
  - Routing (sigmoid gate + group-limited top-k) and token dispatch/combine
    indices are computed on host in fp32 numpy (replicating the reference
    semantics exactly, ~0.1% of total FLOPs).
  - The 32 experts are bin-packed onto 8 cores (4 slots/core) by measured
    load, with per-slot static capacities padded to multiples of 128.
  - Each core runs a Bass/Tile kernel: per slot s,
        gT = silu(w1_s^T @ bufT_s)        [I, Cs]
        hT = gT * (w3_s^T @ bufT_s)       [I, Cs]
        y  = hT^T-form @ w2_s             [Cs, H]
    All matmuls in bf16 with fp32 PSUM accumulation.
  - Combine (gather + weighted sum over the K=8 routes) happens on host.
"""

import math

import ml_dtypes
import numpy as np

import concourse.bass as bass
import concourse.mybir as mybir
import concourse.tile as tile
from concourse import bacc
from concourse.bass_utils import run_bass_kernel_spmd
from concourse.kernels.tile_matmul import (
    composable_matmul_tile_kernel,
    dma_from_dram_kxm,
    dma_from_dram_kxn,
    dma_from_dram_mxn,
    dma_to_dram_mxn,
    k_pool_min_bufs,
    scalar_copyback,
)

# MoE config (matches the reference)
N = 2048
H = 2048
I = 1024
E = 32
K = 8
G = 8
KG = 4
C = 1024
SCALE = 2.5

M_CORES = 8
SLOTS = E // M_CORES  # 4 experts per core

BF16 = ml_dtypes.bfloat16


def _route(x, w_gate, gate_bias):
    """fp32 numpy replication of the reference gate."""
    scores = 1.0 / (1.0 + np.exp(-(x @ w_gate), dtype=np.float32))  # [N, E]
    sb = scores + gate_bias
    grp = sb.reshape(N, G, E // G)
    top2 = -np.sort(-grp, axis=-1)[..., :2]
    gscore = top2.sum(-1)  # [N, G]
    gidx = np.argsort(-gscore, axis=-1, kind="stable")[:, :KG]
    gmask = np.zeros((N, G), bool)
    gmask[np.arange(N)[:, None], gidx] = True
    emask = np.repeat(gmask, E // G, axis=1)
    masked = np.where(emask, sb, -np.inf)
    eidx = np.argsort(-masked, axis=-1, kind="stable")[:, :K]  # [N, K]
    w = np.take_along_axis(scores, eidx, axis=1)
    w = w / w.sum(-1, keepdims=True) * SCALE
    return eidx, w.astype(np.float32)


def _dispatch_indices(eidx):
    """Per-route slot positions, replicating the reference capacity rule."""
    flat_e = eidx.reshape(-1)  # [N*K], token-major arrival order
    tok = np.repeat(np.arange(N), K)
    order = np.argsort(flat_e, kind="stable")
    counts = np.bincount(flat_e, minlength=E)
    starts = np.concatenate([[0], np.cumsum(counts)[:-1]])
    pos_sorted = np.arange(N * K) - np.repeat(starts, counts)
    pos = np.empty(N * K, np.int64)
    pos[order] = pos_sorted
    valid = pos < C
    return flat_e, tok, pos, valid, counts


def _assign_experts(counts):
    """Bin-pack experts: slot s takes load-ranks [8s, 8s+8), snaked across cores."""
    loads = np.minimum(counts, C)
    rank = np.argsort(-loads, kind="stable")
    expert_core = np.empty(E, np.int64)
    expert_slot = np.empty(E, np.int64)
    caps = []
    for s in range(SLOTS):
        grp = rank[M_CORES * s : M_CORES * (s + 1)]
        cores = range(M_CORES) if s % 2 == 0 else range(M_CORES - 1, -1, -1)
        for c, e in zip(cores, grp):
            expert_core[e] = c
            expert_slot[e] = s
        caps.append(int(math.ceil(max(int(loads[e]) for e in grp) / 128.0) * 128))
    offs = np.concatenate([[0], np.cumsum(caps)]).astype(np.int64)
    return expert_core, expert_slot, caps, offs


def _memo_producer(producer):
    memo = {}

    def _p(nc, md):
        key = (md.k_batch_idx, md.k_tile_idx, md.n_batch_idx, md.n_tile_idx)
        if key not in memo:
            memo[key] = producer(nc, md)
        return memo[key]

    return _p


def _slot_graph(ctx, tc, s, Cs, w1_ap, w3_ap, w2_ap, buf_ap, y_ap):
    """One expert slot: gT = silu(w1^T buf), hT = gT * (w3^T buf), y = hT^T-form w2.
    gT/hT stay resident in SBUF; buf and w2 are DMA'd once (memoized producers)."""
    nc = tc.nc
    P = 128
    ISUB = I // P  # 8
    Cp = int(math.ceil(Cs / 512.0) * 512)  # pad free dim so n-tile views stay in range

    spool = ctx.enter_context(tc.tile_pool(name=f"sb{s}", bufs=1))
    gt_sb = spool.tile([P, ISUB, Cp], mybir.dt.bfloat16, name=f"gt{s}")
    ht_sb = spool.tile([P, ISUB, Cp], mybir.dt.bfloat16, name=f"ht{s}")

    from concourse.kernels.tile_matmul import ShapeInfo

    n_buf_tiles = int(math.ceil(Cs / 512.0)) * (H // 512)  # live buf tiles per slot
    bpool = ctx.enter_context(tc.tile_pool(name=f"bf{s}", bufs=n_buf_tiles + 1))
    buf_producer_raw, buf_shape = dma_from_dram_kxn(bpool, buf_ap)
    buf_producer = _memo_producer(buf_producer_raw)

    w1_pool = ctx.enter_context(tc.tile_pool(name=f"w1p{s}", bufs=8))
    w3_pool = ctx.enter_context(tc.tile_pool(name=f"w3p{s}", bufs=8))
    w1_producer, w1_shape = dma_from_dram_kxm(w1_pool, w1_ap)
    w3_producer, _ = dma_from_dram_kxm(w3_pool, w3_ap)

    def gt_view(md):
        return gt_sb[:, bass.ts(md.m_tile_idx, md.m_subtiles),
                     bass.ds(md.n_tile_idx * md.n_tile, md.n_tile)]

    def ht_view(md):
        return ht_sb[:, bass.ts(md.m_tile_idx, md.m_subtiles),
                     bass.ds(md.n_tile_idx * md.n_tile, md.n_tile)]

    def silu_reduce(nc_, psum, sbuf, md):
        nc_.scalar.activation(sbuf[:], psum[:], mybir.ActivationFunctionType.Silu)

    def mul_reduce(nc_, psum, sbuf, md):
        g = gt_sb[:, bass.ds(md.m_tile_idx * md.m_subtiles + md.m_subtile_idx, 1),
                  bass.ds(md.n_tile_idx * md.n_tile + md.n_subtile_idx * md.n_subtile,
                          md.n_subtile)]
        nc_.vector.tensor_mul(out=sbuf[:], in0=psum[:], in1=g)

    def noop_consumer(nc_, sbuf, md):
        pass

    # mm1a: gT = silu(w1^T @ buf)
    composable_matmul_tile_kernel(
        tc=tc, kxm_shape=w1_shape, kxn_shape=buf_shape, output_type=mybir.dt.bfloat16,
        kxm_producer=w1_producer, kxn_producer=buf_producer,
        mxn_consumer=noop_consumer, mxn_subtile_reducer=silu_reduce,
        mxn_subtile_producer=lambda nc_, md: gt_view(md), psum_n_bufs=1,
    )
    # mm1b: hT = gT * (w3^T @ buf)
    composable_matmul_tile_kernel(
        tc=tc, kxm_shape=w1_shape, kxn_shape=buf_shape, output_type=mybir.dt.bfloat16,
        kxm_producer=w3_producer, kxn_producer=buf_producer,
        mxn_consumer=noop_consumer, mxn_subtile_reducer=mul_reduce,
        mxn_subtile_producer=lambda nc_, md: ht_view(md), psum_n_bufs=1,
    )

    # mm2: y = hT^T-form @ w2 ; kxm tiles are views of ht_sb
    ht_shape = ShapeInfo(pdims=((P, ISUB),), fdims=(Cs,))

    def ht_kxm_producer(nc_, md):
        return ht_sb[:, bass.ds(md.k_tile_idx * md.k_subtiles, md.k_subtiles),
                     bass.ds(md.m_tile_idx * md.m_tile, md.m_tile)]

    w2_pool = ctx.enter_context(tc.tile_pool(name=f"w2p{s}", bufs=9))
    w2_producer_raw, w2_shape = dma_from_dram_kxn(w2_pool, w2_ap)
    w2_producer = _memo_producer(w2_producer_raw)
    composable_matmul_tile_kernel(
        tc=tc, kxm_shape=ht_shape, kxn_shape=w2_shape, output_type=mybir.dt.float32,
        kxm_producer=ht_kxm_producer, kxn_producer=w2_producer,
        mxn_consumer=dma_to_dram_mxn(y_ap),
        mxn_subtile_reducer=scalar_copyback(), psum_n_bufs=1,
    )


def _build_graph(caps, offs):
    from contextlib import ExitStack

    CT = int(offs[-1])
    nc = bacc.Bacc(None, target_bir_lowering=False, debug=False)
    w1_p = nc.declare_dram_parameter("w1s", [SLOTS, H, I], mybir.dt.bfloat16, isOutput=False)
    w3_p = nc.declare_dram_parameter("w3s", [SLOTS, H, I], mybir.dt.bfloat16, isOutput=False)
    w2_p = nc.declare_dram_parameter("w2s", [SLOTS, I, H], mybir.dt.bfloat16, isOutput=False)
    buf_p = nc.declare_dram_parameter("buft", [H, CT], mybir.dt.bfloat16, isOutput=False)
    y_p = nc.declare_dram_parameter("y", [CT, H], mybir.dt.float32, isOutput=True)

    with tile.TileContext(nc) as tc:
        for s in range(SLOTS):
            sl = bass.ds(int(offs[s]), caps[s])
            with ExitStack() as sctx:
                _slot_graph(sctx, tc, s, caps[s], w1_p[s], w3_p[s], w2_p[s],
                            buf_p[:, sl], y_p[sl, :])
    nc.compile()
    return nc


_GRAPH_CACHE = {}


def kernel(x, w_gate, gate_bias, w1, w3, w2):
    x = np.asarray(x, np.float32)
    w_gate = np.asarray(w_gate, np.float32)
    gate_bias = np.asarray(gate_bias, np.float32)

    eidx, w = _route(x, w_gate, gate_bias)
    flat_e, tok, pos, valid, counts = _dispatch_indices(eidx)
    expert_core, expert_slot, caps, offs = _assign_experts(counts)
    CT = int(offs[-1])

    # ---- stage per-core inputs ----
    bufT = np.zeros((M_CORES, H, CT), BF16)
    expert_tokens = {}
    for e in range(E):
        m = (flat_e == e) & valid
        toks = tok[m]  # arrival order == slot positions 0..len-1
        expert_tokens[e] = toks
        c, s = int(expert_core[e]), int(expert_slot[e])
        o = int(offs[s])
        bufT[c][:, o : o + len(toks)] = x[toks].astype(BF16).T

    w1b = np.asarray(w1, np.float32).astype(BF16)
    w3b = np.asarray(w3, np.float32).astype(BF16)
    w2b = np.asarray(w2, np.float32).astype(BF16)
    in_maps = []
    for c in range(M_CORES):
        # experts of core c in slot order
        es = [int(np.where((expert_core == c) & (expert_slot == s))[0][0]) for s in range(SLOTS)]
        in_maps.append({
            "w1s": np.ascontiguousarray(w1b[es]),
            "w3s": np.ascontiguousarray(w3b[es]),
            "w2s": np.ascontiguousarray(w2b[es]),
            "buft": bufT[c],
        })

    # ---- build + run ----
    key = tuple(caps)
    if key not in _GRAPH_CACHE:
        _GRAPH_CACHE[key] = _build_graph(caps, offs)
    nc = _GRAPH_CACHE[key]
    res = run_bass_kernel_spmd(nc, in_maps, core_ids=list(range(M_CORES)))
    ys = np.stack([res.results[c]["y"] for c in range(M_CORES)])  # [8, CT, H] f32

    # ---- combine on host ----
    ycat = ys.reshape(M_CORES * CT, H)
    core_of = expert_core[flat_e]
    slot_of = expert_slot[flat_e]
    col = offs[slot_of] + pos
    row = core_of * CT + col
    row = np.where(valid, row, 0)
    wf = np.where(valid, w.reshape(-1), 0.0).astype(np.float32)
    contrib = ycat[row] * wf[:, None]  # [N*K, H]
    out = contrib.reshape(N, K, H).sum(axis=1)
    return out.astype(np.float32)


# revision 15
# speedup vs baseline: 1.3014x; 1.1231x over previous
"""DeepSeek-style MoE layer on 8 Trainium2 NeuronCores, expert-parallel.

Strategy:
  - Routing (sigmoid gate + group-limited top-k) and token dispatch/combine
    indices are computed on host in fp32 numpy (replicating the reference
    semantics exactly, ~0.1% of total FLOPs).
  - The 32 experts are bin-packed onto 8 cores (4 slots/core) by measured
    load, with per-slot static capacities padded to multiples of 128.
  - Each core runs a Bass/Tile kernel: per slot s,
        gT = silu(w1_s^T @ bufT_s)        [I, Cs]
        hT = gT * (w3_s^T @ bufT_s)       [I, Cs]
        y  = hT^T-form @ w2_s             [Cs, H]
    All matmuls in bf16 with fp32 PSUM accumulation.
  - Combine (gather + weighted sum over the K=8 routes) happens on host.
"""

import math

import ml_dtypes
import numpy as np

import concourse.bass as bass
import concourse.mybir as mybir
import concourse.tile as tile
from concourse import bacc
from concourse.bass_utils import run_bass_kernel_spmd
from concourse.kernels.tile_matmul import (
    composable_matmul_tile_kernel,
    dma_from_dram_kxm,
    dma_from_dram_kxn,
    dma_from_dram_mxn,
    dma_to_dram_mxn,
    k_pool_min_bufs,
    scalar_copyback,
)

# MoE config (matches the reference)
N = 2048
H = 2048
I = 1024
E = 32
K = 8
G = 8
KG = 4
C = 1024
SCALE = 2.5

M_CORES = 8
SLOTS = E // M_CORES  # 4 experts per core

BF16 = ml_dtypes.bfloat16


def _route(x, w_gate, gate_bias):
    """fp32 numpy replication of the reference gate."""
    scores = 1.0 / (1.0 + np.exp(-(x @ w_gate), dtype=np.float32))  # [N, E]
    sb = scores + gate_bias
    grp = sb.reshape(N, G, E // G)
    top2 = -np.sort(-grp, axis=-1)[..., :2]
    gscore = top2.sum(-1)  # [N, G]
    gidx = np.argsort(-gscore, axis=-1, kind="stable")[:, :KG]
    gmask = np.zeros((N, G), bool)
    gmask[np.arange(N)[:, None], gidx] = True
    emask = np.repeat(gmask, E // G, axis=1)
    masked = np.where(emask, sb, -np.inf)
    eidx = np.argsort(-masked, axis=-1, kind="stable")[:, :K]  # [N, K]
    w = np.take_along_axis(scores, eidx, axis=1)
    w = w / w.sum(-1, keepdims=True) * SCALE
    return eidx, w.astype(np.float32)


def _dispatch_indices(eidx):
    """Per-route slot positions, replicating the reference capacity rule."""
    flat_e = eidx.reshape(-1)  # [N*K], token-major arrival order
    tok = np.repeat(np.arange(N), K)
    order = np.argsort(flat_e, kind="stable")
    counts = np.bincount(flat_e, minlength=E)
    starts = np.concatenate([[0], np.cumsum(counts)[:-1]])
    pos_sorted = np.arange(N * K) - np.repeat(starts, counts)
    pos = np.empty(N * K, np.int64)
    pos[order] = pos_sorted
    valid = pos < C
    return flat_e, tok, pos, valid, counts


def _assign_experts(counts):
    """Bin-pack experts: slot s takes load-ranks [8s, 8s+8), snaked across cores."""
    loads = np.minimum(counts, C)
    rank = np.argsort(-loads, kind="stable")
    expert_core = np.empty(E, np.int64)
    expert_slot = np.empty(E, np.int64)
    caps = []
    for s in range(SLOTS):
        grp = rank[M_CORES * s : M_CORES * (s + 1)]
        cores = range(M_CORES) if s % 2 == 0 else range(M_CORES - 1, -1, -1)
        for c, e in zip(cores, grp):
            expert_core[e] = c
            expert_slot[e] = s
        caps.append(int(math.ceil(max(int(loads[e]) for e in grp) / 128.0) * 128))
    offs = np.concatenate([[0], np.cumsum(caps)]).astype(np.int64)
    return expert_core, expert_slot, caps, offs


def _memo_producer(producer):
    memo = {}

    def _p(nc, md):
        key = (md.k_batch_idx, md.k_tile_idx, md.n_batch_idx, md.n_tile_idx)
        if key not in memo:
            memo[key] = producer(nc, md)
        return memo[key]

    return _p


def _slot_graph(ctx, tc, s, Cs, w1_ap, w3_ap, w2_ap, buf_ap, y_ap):
    """One expert slot: gT = silu(w1^T buf), hT = gT * (w3^T buf), y = hT^T-form w2.
    gT/hT stay resident in SBUF; buf and w2 are DMA'd once (memoized producers)."""
    nc = tc.nc
    P = 128
    ISUB = I // P  # 8
    n_tiles = int(math.ceil(Cs / 512.0))
    nt = Cs // n_tiles  # exact n-tile size for mm1: 768->384, 640->320, 512->512
    assert nt * n_tiles == Cs and nt % 64 == 0
    Cp = Cs  # exact tiling: no padded columns needed

    spool = ctx.enter_context(tc.tile_pool(name=f"sb{s}", bufs=1))
    gt_sb = spool.tile([P, ISUB, Cp], mybir.dt.bfloat16, name=f"gt{s}")
    ht_sb = spool.tile([P, ISUB, Cp], mybir.dt.bfloat16, name=f"ht{s}")

    from concourse.kernels.tile_matmul import ShapeInfo

    n_buf_tiles = int(math.ceil(Cs / 512.0)) * (H // 512)  # live buf tiles per slot
    bpool = ctx.enter_context(tc.tile_pool(name=f"bf{s}", bufs=n_buf_tiles + 1))
    buf_producer_raw, buf_shape = dma_from_dram_kxn(bpool, buf_ap)
    buf_producer = _memo_producer(buf_producer_raw)

    w1_pool = ctx.enter_context(tc.tile_pool(name=f"w1p{s}", bufs=8))
    w3_pool = ctx.enter_context(tc.tile_pool(name=f"w3p{s}", bufs=8))
    w1_producer, w1_shape = dma_from_dram_kxm(w1_pool, w1_ap)
    w3_producer, _ = dma_from_dram_kxm(w3_pool, w3_ap)

    def gt_view(md):
        return gt_sb[:, bass.ts(md.m_tile_idx, md.m_subtiles),
                     bass.ds(md.n_tile_idx * md.n_tile, md.n_tile)]

    def ht_view(md):
        return ht_sb[:, bass.ts(md.m_tile_idx, md.m_subtiles),
                     bass.ds(md.n_tile_idx * md.n_tile, md.n_tile)]

    def silu_reduce(nc_, psum, sbuf, md):
        nc_.scalar.activation(sbuf[:], psum[:], mybir.ActivationFunctionType.Silu)

    def mul_reduce(nc_, psum, sbuf, md):
        g = gt_sb[:, bass.ds(md.m_tile_idx * md.m_subtiles + md.m_subtile_idx, 1),
                  bass.ds(md.n_tile_idx * md.n_tile + md.n_subtile_idx * md.n_subtile,
                          md.n_subtile)]
        nc_.vector.tensor_mul(out=sbuf[:], in0=psum[:], in1=g)

    def noop_consumer(nc_, sbuf, md):
        pass

    # mm1a: gT = silu(w1^T @ buf)
    composable_matmul_tile_kernel(
        tc=tc, kxm_shape=w1_shape, kxn_shape=buf_shape, output_type=mybir.dt.bfloat16,
        kxm_producer=w1_producer, kxn_producer=buf_producer,
        mxn_consumer=noop_consumer, mxn_subtile_reducer=silu_reduce,
        mxn_subtile_producer=lambda nc_, md: gt_view(md), psum_n_bufs=1,
        MAX_TILE_SIZE=nt,
    )
    # mm1b: hT = gT * (w3^T @ buf)
    composable_matmul_tile_kernel(
        tc=tc, kxm_shape=w1_shape, kxn_shape=buf_shape, output_type=mybir.dt.bfloat16,
        kxm_producer=w3_producer, kxn_producer=buf_producer,
        mxn_consumer=noop_consumer, mxn_subtile_reducer=mul_reduce,
        mxn_subtile_producer=lambda nc_, md: ht_view(md), psum_n_bufs=1,
        MAX_TILE_SIZE=nt,
    )

    # mm2: y = hT^T-form @ w2 ; kxm tiles are views of ht_sb
    ht_shape = ShapeInfo(pdims=((P, ISUB),), fdims=(Cs,))

    def ht_kxm_producer(nc_, md):
        return ht_sb[:, bass.ds(md.k_tile_idx * md.k_subtiles, md.k_subtiles),
                     bass.ds(md.m_tile_idx * md.m_tile, md.m_tile)]

    w2_pool = ctx.enter_context(tc.tile_pool(name=f"w2p{s}", bufs=9))
    w2_producer_raw, w2_shape = dma_from_dram_kxn(w2_pool, w2_ap)
    w2_producer = _memo_producer(w2_producer_raw)
    composable_matmul_tile_kernel(
        tc=tc, kxm_shape=ht_shape, kxn_shape=w2_shape, output_type=mybir.dt.float32,
        kxm_producer=ht_kxm_producer, kxn_producer=w2_producer,
        mxn_consumer=dma_to_dram_mxn(y_ap),
        mxn_subtile_reducer=scalar_copyback(), psum_n_bufs=1,
    )


def _build_graph(caps, offs):
    from contextlib import ExitStack

    CT = int(offs[-1])
    nc = bacc.Bacc(None, target_bir_lowering=False, debug=False)
    w1_p = nc.declare_dram_parameter("w1s", [SLOTS, H, I], mybir.dt.bfloat16, isOutput=False)
    w3_p = nc.declare_dram_parameter("w3s", [SLOTS, H, I], mybir.dt.bfloat16, isOutput=False)
    w2_p = nc.declare_dram_parameter("w2s", [SLOTS, I, H], mybir.dt.bfloat16, isOutput=False)
    buf_p = nc.declare_dram_parameter("buft", [H, CT], mybir.dt.bfloat16, isOutput=False)
    y_p = nc.declare_dram_parameter("y", [CT, H], mybir.dt.float32, isOutput=True)

    with tile.TileContext(nc) as tc:
        for s in range(SLOTS):
            sl = bass.ds(int(offs[s]), caps[s])
            with ExitStack() as sctx:
                _slot_graph(sctx, tc, s, caps[s], w1_p[s], w3_p[s], w2_p[s],
                            buf_p[:, sl], y_p[sl, :])
    nc.compile()
    return nc


_GRAPH_CACHE = {}


def kernel(x, w_gate, gate_bias, w1, w3, w2):
    x = np.asarray(x, np.float32)
    w_gate = np.asarray(w_gate, np.float32)
    gate_bias = np.asarray(gate_bias, np.float32)

    eidx, w = _route(x, w_gate, gate_bias)
    flat_e, tok, pos, valid, counts = _dispatch_indices(eidx)
    expert_core, expert_slot, caps, offs = _assign_experts(counts)
    CT = int(offs[-1])

    # ---- stage per-core inputs ----
    bufT = np.zeros((M_CORES, H, CT), BF16)
    expert_tokens = {}
    for e in range(E):
        m = (flat_e == e) & valid
        toks = tok[m]  # arrival order == slot positions 0..len-1
        expert_tokens[e] = toks
        c, s = int(expert_core[e]), int(expert_slot[e])
        o = int(offs[s])
        bufT[c][:, o : o + len(toks)] = x[toks].astype(BF16).T

    w1b = np.asarray(w1, np.float32).astype(BF16)
    w3b = np.asarray(w3, np.float32).astype(BF16)
    w2b = np.asarray(w2, np.float32).astype(BF16)
    in_maps = []
    for c in range(M_CORES):
        # experts of core c in slot order
        es = [int(np.where((expert_core == c) & (expert_slot == s))[0][0]) for s in range(SLOTS)]
        in_maps.append({
            "w1s": np.ascontiguousarray(w1b[es]),
            "w3s": np.ascontiguousarray(w3b[es]),
            "w2s": np.ascontiguousarray(w2b[es]),
            "buft": bufT[c],
        })

    # ---- build + run ----
    key = tuple(caps)
    if key not in _GRAPH_CACHE:
        _GRAPH_CACHE[key] = _build_graph(caps, offs)
    nc = _GRAPH_CACHE[key]
    res = run_bass_kernel_spmd(nc, in_maps, core_ids=list(range(M_CORES)))
    ys = np.stack([res.results[c]["y"] for c in range(M_CORES)])  # [8, CT, H] f32

    # ---- combine on host ----
    ycat = ys.reshape(M_CORES * CT, H)
    core_of = expert_core[flat_e]
    slot_of = expert_slot[flat_e]
    col = offs[slot_of] + pos
    row = core_of * CT + col
    row = np.where(valid, row, 0)
    wf = np.where(valid, w.reshape(-1), 0.0).astype(np.float32)
    contrib = ycat[row] * wf[:, None]  # [N*K, H]
    out = contrib.reshape(N, K, H).sum(axis=1)
    return out.astype(np.float32)


# revision 16
# speedup vs baseline: 1.3032x; 1.0014x over previous
"""DeepSeek-style MoE layer on 8 Trainium2 NeuronCores, expert-parallel.

Strategy:
  - Routing (sigmoid gate + group-limited top-k) and token dispatch/combine
    indices are computed on host in fp32 numpy (replicating the reference
    semantics exactly, ~0.1% of total FLOPs).
  - The 32 experts are bin-packed onto 8 cores (4 slots/core) by measured
    load, with per-slot static capacities padded to multiples of 128.
  - Each core runs a Bass/Tile kernel: per slot s,
        gT = silu(w1_s^T @ bufT_s)        [I, Cs]
        hT = gT * (w3_s^T @ bufT_s)       [I, Cs]
        y  = hT^T-form @ w2_s             [Cs, H]
    All matmuls in bf16 with fp32 PSUM accumulation.
  - Combine (gather + weighted sum over the K=8 routes) happens on host.
"""

import math

import ml_dtypes
import numpy as np

import concourse.bass as bass
import concourse.mybir as mybir
import concourse.tile as tile
from concourse import bacc
from concourse.bass_utils import run_bass_kernel_spmd
from concourse.kernels.tile_matmul import (
    composable_matmul_tile_kernel,
    dma_from_dram_kxm,
    dma_from_dram_kxn,
    dma_from_dram_mxn,
    dma_to_dram_mxn,
    k_pool_min_bufs,
    scalar_copyback,
)

# MoE config (matches the reference)
N = 2048
H = 2048
I = 1024
E = 32
K = 8
G = 8
KG = 4
C = 1024
SCALE = 2.5

M_CORES = 8
SLOTS = E // M_CORES  # 4 experts per core

BF16 = ml_dtypes.bfloat16


def _route(x, w_gate, gate_bias):
    """fp32 numpy replication of the reference gate."""
    scores = 1.0 / (1.0 + np.exp(-(x @ w_gate), dtype=np.float32))  # [N, E]
    sb = scores + gate_bias
    grp = sb.reshape(N, G, E // G)
    top2 = -np.sort(-grp, axis=-1)[..., :2]
    gscore = top2.sum(-1)  # [N, G]
    gidx = np.argsort(-gscore, axis=-1, kind="stable")[:, :KG]
    gmask = np.zeros((N, G), bool)
    gmask[np.arange(N)[:, None], gidx] = True
    emask = np.repeat(gmask, E // G, axis=1)
    masked = np.where(emask, sb, -np.inf)
    eidx = np.argsort(-masked, axis=-1, kind="stable")[:, :K]  # [N, K]
    w = np.take_along_axis(scores, eidx, axis=1)
    w = w / w.sum(-1, keepdims=True) * SCALE
    return eidx, w.astype(np.float32)


def _dispatch_indices(eidx):
    """Per-route slot positions, replicating the reference capacity rule."""
    flat_e = eidx.reshape(-1)  # [N*K], token-major arrival order
    tok = np.repeat(np.arange(N), K)
    order = np.argsort(flat_e, kind="stable")
    counts = np.bincount(flat_e, minlength=E)
    starts = np.concatenate([[0], np.cumsum(counts)[:-1]])
    pos_sorted = np.arange(N * K) - np.repeat(starts, counts)
    pos = np.empty(N * K, np.int64)
    pos[order] = pos_sorted
    valid = pos < C
    return flat_e, tok, pos, valid, counts


def _assign_experts(counts):
    """Bin-pack experts: slot s takes load-ranks [8s, 8s+8), snaked across cores."""
    loads = np.minimum(counts, C)
    rank = np.argsort(-loads, kind="stable")
    expert_core = np.empty(E, np.int64)
    expert_slot = np.empty(E, np.int64)
    caps = []
    for s in range(SLOTS):
        grp = rank[M_CORES * s : M_CORES * (s + 1)]
        cores = range(M_CORES) if s % 2 == 0 else range(M_CORES - 1, -1, -1)
        for c, e in zip(cores, grp):
            expert_core[e] = c
            expert_slot[e] = s
        caps.append(int(math.ceil(max(int(loads[e]) for e in grp) / 128.0) * 128))
    offs = np.concatenate([[0], np.cumsum(caps)]).astype(np.int64)
    return expert_core, expert_slot, caps, offs


def _memo_producer(producer):
    memo = {}

    def _p(nc, md):
        key = (md.k_batch_idx, md.k_tile_idx, md.n_batch_idx, md.n_tile_idx)
        if key not in memo:
            memo[key] = producer(nc, md)
        return memo[key]

    return _p


def _slot_graph(ctx, tc, s, Cs, w1_ap, w3_ap, w2_ap, buf_ap, y_ap):
    """One expert slot: gT = silu(w1^T buf), hT = gT * (w3^T buf), y = hT^T-form w2.
    gT/hT stay resident in SBUF; buf and w2 are DMA'd once (memoized producers)."""
    nc = tc.nc
    P = 128
    ISUB = I // P  # 8
    n_tiles = int(math.ceil(Cs / 512.0))
    nt = Cs // n_tiles  # exact n-tile size for mm1: 768->384, 640->320, 512->512
    assert nt * n_tiles == Cs and nt % 64 == 0
    Cp = Cs  # exact tiling: no padded columns needed

    spool = ctx.enter_context(tc.tile_pool(name=f"sb{s}", bufs=1))
    gt_sb = spool.tile([P, ISUB, Cp], mybir.dt.bfloat16, name=f"gt{s}")
    ht_sb = spool.tile([P, ISUB, Cp], mybir.dt.bfloat16, name=f"ht{s}")

    from concourse.kernels.tile_matmul import ShapeInfo

    n_buf_tiles = int(math.ceil(Cs / 512.0)) * (H // 512)  # live buf tiles per slot
    bpool = ctx.enter_context(tc.tile_pool(name=f"bf{s}", bufs=n_buf_tiles + 2))
    buf_producer_raw, buf_shape = dma_from_dram_kxn(bpool, buf_ap)
    buf_producer = _memo_producer(buf_producer_raw)

    w1_pool = ctx.enter_context(tc.tile_pool(name=f"w1p{s}", bufs=12))
    w3_pool = ctx.enter_context(tc.tile_pool(name=f"w3p{s}", bufs=12))
    w1_producer, w1_shape = dma_from_dram_kxm(w1_pool, w1_ap)
    w3_producer, _ = dma_from_dram_kxm(w3_pool, w3_ap)

    def gt_view(md):
        return gt_sb[:, bass.ts(md.m_tile_idx, md.m_subtiles),
                     bass.ds(md.n_tile_idx * md.n_tile, md.n_tile)]

    def ht_view(md):
        return ht_sb[:, bass.ts(md.m_tile_idx, md.m_subtiles),
                     bass.ds(md.n_tile_idx * md.n_tile, md.n_tile)]

    def silu_reduce(nc_, psum, sbuf, md):
        nc_.scalar.activation(sbuf[:], psum[:], mybir.ActivationFunctionType.Silu)

    def mul_reduce(nc_, psum, sbuf, md):
        g = gt_sb[:, bass.ds(md.m_tile_idx * md.m_subtiles + md.m_subtile_idx, 1),
                  bass.ds(md.n_tile_idx * md.n_tile + md.n_subtile_idx * md.n_subtile,
                          md.n_subtile)]
        nc_.vector.tensor_mul(out=sbuf[:], in0=psum[:], in1=g)

    def noop_consumer(nc_, sbuf, md):
        pass

    # mm1a: gT = silu(w1^T @ buf)
    composable_matmul_tile_kernel(
        tc=tc, kxm_shape=w1_shape, kxn_shape=buf_shape, output_type=mybir.dt.bfloat16,
        kxm_producer=w1_producer, kxn_producer=buf_producer,
        mxn_consumer=noop_consumer, mxn_subtile_reducer=silu_reduce,
        mxn_subtile_producer=lambda nc_, md: gt_view(md), psum_n_bufs=1,
        MAX_TILE_SIZE=nt,
    )
    # mm1b: hT = gT * (w3^T @ buf)
    composable_matmul_tile_kernel(
        tc=tc, kxm_shape=w1_shape, kxn_shape=buf_shape, output_type=mybir.dt.bfloat16,
        kxm_producer=w3_producer, kxn_producer=buf_producer,
        mxn_consumer=noop_consumer, mxn_subtile_reducer=mul_reduce,
        mxn_subtile_producer=lambda nc_, md: ht_view(md), psum_n_bufs=1,
        MAX_TILE_SIZE=nt,
    )

    # mm2: y = hT^T-form @ w2 ; kxm tiles are views of ht_sb
    ht_shape = ShapeInfo(pdims=((P, ISUB),), fdims=(Cs,))

    def ht_kxm_producer(nc_, md):
        return ht_sb[:, bass.ds(md.k_tile_idx * md.k_subtiles, md.k_subtiles),
                     bass.ds(md.m_tile_idx * md.m_tile, md.m_tile)]

    w2_pool = ctx.enter_context(tc.tile_pool(name=f"w2p{s}", bufs=9))
    w2_producer_raw, w2_shape = dma_from_dram_kxn(w2_pool, w2_ap)
    w2_producer = _memo_producer(w2_producer_raw)
    composable_matmul_tile_kernel(
        tc=tc, kxm_shape=ht_shape, kxn_shape=w2_shape, output_type=mybir.dt.float32,
        kxm_producer=ht_kxm_producer, kxn_producer=w2_producer,
        mxn_consumer=dma_to_dram_mxn(y_ap),
        mxn_subtile_reducer=scalar_copyback(), psum_n_bufs=1,
    )


def _build_graph(caps, offs):
    from contextlib import ExitStack

    CT = int(offs[-1])
    nc = bacc.Bacc(None, target_bir_lowering=False, debug=False)
    w1_p = nc.declare_dram_parameter("w1s", [SLOTS, H, I], mybir.dt.bfloat16, isOutput=False)
    w3_p = nc.declare_dram_parameter("w3s", [SLOTS, H, I], mybir.dt.bfloat16, isOutput=False)
    w2_p = nc.declare_dram_parameter("w2s", [SLOTS, I, H], mybir.dt.bfloat16, isOutput=False)
    buf_p = nc.declare_dram_parameter("buft", [H, CT], mybir.dt.bfloat16, isOutput=False)
    y_p = nc.declare_dram_parameter("y", [CT, H], mybir.dt.float32, isOutput=True)

    with tile.TileContext(nc) as tc:
        for s in range(SLOTS):
            sl = bass.ds(int(offs[s]), caps[s])
            with ExitStack() as sctx:
                _slot_graph(sctx, tc, s, caps[s], w1_p[s], w3_p[s], w2_p[s],
                            buf_p[:, sl], y_p[sl, :])
    nc.compile()
    return nc


_GRAPH_CACHE = {}


def kernel(x, w_gate, gate_bias, w1, w3, w2):
    x = np.asarray(x, np.float32)
    w_gate = np.asarray(w_gate, np.float32)
    gate_bias = np.asarray(gate_bias, np.float32)

    eidx, w = _route(x, w_gate, gate_bias)
    flat_e, tok, pos, valid, counts = _dispatch_indices(eidx)
    expert_core, expert_slot, caps, offs = _assign_experts(counts)
    CT = int(offs[-1])

    # ---- stage per-core inputs ----
    bufT = np.zeros((M_CORES, H, CT), BF16)
    expert_tokens = {}
    for e in range(E):
        m = (flat_e == e) & valid
        toks = tok[m]  # arrival order == slot positions 0..len-1
        expert_tokens[e] = toks
        c, s = int(expert_core[e]), int(expert_slot[e])
        o = int(offs[s])
        bufT[c][:, o : o + len(toks)] = x[toks].astype(BF16).T

    w1b = np.asarray(w1, np.float32).astype(BF16)
    w3b = np.asarray(w3, np.float32).astype(BF16)
    w2b = np.asarray(w2, np.float32).astype(BF16)
    in_maps = []
    for c in range(M_CORES):
        # experts of core c in slot order
        es = [int(np.where((expert_core == c) & (expert_slot == s))[0][0]) for s in range(SLOTS)]
        in_maps.append({
            "w1s": np.ascontiguousarray(w1b[es]),
            "w3s": np.ascontiguousarray(w3b[es]),
            "w2s": np.ascontiguousarray(w2b[es]),
            "buft": bufT[c],
        })

    # ---- build + run ----
    key = tuple(caps)
    if key not in _GRAPH_CACHE:
        _GRAPH_CACHE[key] = _build_graph(caps, offs)
    nc = _GRAPH_CACHE[key]
    res = run_bass_kernel_spmd(nc, in_maps, core_ids=list(range(M_CORES)))
    ys = np.stack([res.results[c]["y"] for c in range(M_CORES)])  # [8, CT, H] f32

    # ---- combine on host ----
    ycat = ys.reshape(M_CORES * CT, H)
    core_of = expert_core[flat_e]
    slot_of = expert_slot[flat_e]
    col = offs[slot_of] + pos
    row = core_of * CT + col
    row = np.where(valid, row, 0)
    wf = np.where(valid, w.reshape(-1), 0.0).astype(np.float32)
    contrib = ycat[row] * wf[:, None]  # [N*K, H]
    out = contrib.reshape(N, K, H).sum(axis=1)
    return out.astype(np.float32)


# revision 17
# speedup vs baseline: 1.3170x; 1.0106x over previous
"""DeepSeek-style MoE layer on 8 Trainium2 NeuronCores, expert-parallel.

Strategy:
  - Routing (sigmoid gate + group-limited top-k) and token dispatch/combine
    indices are computed on host in fp32 numpy (replicating the reference
    semantics exactly, ~0.1% of total FLOPs).
  - The 32 experts are bin-packed onto 8 cores (4 slots/core) by measured
    load, with per-slot static capacities padded to multiples of 128.
  - Each core runs a Bass/Tile kernel: per slot s,
        gT = silu(w1_s^T @ bufT_s)        [I, Cs]
        hT = gT * (w3_s^T @ bufT_s)       [I, Cs]
        y  = hT^T-form @ w2_s             [Cs, H]
    All matmuls in bf16 with fp32 PSUM accumulation.
  - Combine (gather + weighted sum over the K=8 routes) happens on host.
"""

import math

import ml_dtypes
import numpy as np

import concourse.bass as bass
import concourse.mybir as mybir
import concourse.tile as tile
from concourse import bacc
from concourse.bass_utils import run_bass_kernel_spmd
from concourse.kernels.tile_matmul import (
    composable_matmul_tile_kernel,
    dma_from_dram_kxm,
    dma_from_dram_kxn,
    dma_from_dram_mxn,
    dma_to_dram_mxn,
    k_pool_min_bufs,
    scalar_copyback,
)

# MoE config (matches the reference)
N = 2048
H = 2048
I = 1024
E = 32
K = 8
G = 8
KG = 4
C = 1024
SCALE = 2.5

M_CORES = 8
SLOTS = E // M_CORES  # 4 experts per core

BF16 = ml_dtypes.bfloat16


def _route(x, w_gate, gate_bias):
    """fp32 numpy replication of the reference gate."""
    scores = 1.0 / (1.0 + np.exp(-(x @ w_gate), dtype=np.float32))  # [N, E]
    sb = scores + gate_bias
    grp = sb.reshape(N, G, E // G)
    top2 = -np.sort(-grp, axis=-1)[..., :2]
    gscore = top2.sum(-1)  # [N, G]
    gidx = np.argsort(-gscore, axis=-1, kind="stable")[:, :KG]
    gmask = np.zeros((N, G), bool)
    gmask[np.arange(N)[:, None], gidx] = True
    emask = np.repeat(gmask, E // G, axis=1)
    masked = np.where(emask, sb, -np.inf)
    eidx = np.argsort(-masked, axis=-1, kind="stable")[:, :K]  # [N, K]
    w = np.take_along_axis(scores, eidx, axis=1)
    w = w / w.sum(-1, keepdims=True) * SCALE
    return eidx, w.astype(np.float32)


def _dispatch_indices(eidx):
    """Per-route slot positions, replicating the reference capacity rule."""
    flat_e = eidx.reshape(-1)  # [N*K], token-major arrival order
    tok = np.repeat(np.arange(N), K)
    order = np.argsort(flat_e, kind="stable")
    counts = np.bincount(flat_e, minlength=E)
    starts = np.concatenate([[0], np.cumsum(counts)[:-1]])
    pos_sorted = np.arange(N * K) - np.repeat(starts, counts)
    pos = np.empty(N * K, np.int64)
    pos[order] = pos_sorted
    valid = pos < C
    return flat_e, tok, pos, valid, counts


def _assign_experts(counts):
    """Bin-pack experts: slot s takes load-ranks [8s, 8s+8), snaked across cores."""
    loads = np.minimum(counts, C)
    rank = np.argsort(-loads, kind="stable")
    expert_core = np.empty(E, np.int64)
    expert_slot = np.empty(E, np.int64)
    caps = []
    for s in range(SLOTS):
        grp = rank[M_CORES * s : M_CORES * (s + 1)]
        cores = range(M_CORES) if s % 2 == 0 else range(M_CORES - 1, -1, -1)
        for c, e in zip(cores, grp):
            expert_core[e] = c
            expert_slot[e] = s
        caps.append(int(math.ceil(max(int(loads[e]) for e in grp) / 128.0) * 128))
    offs = np.concatenate([[0], np.cumsum(caps)]).astype(np.int64)
    return expert_core, expert_slot, caps, offs


def _memo_producer(producer):
    memo = {}

    def _p(nc, md):
        key = (md.k_batch_idx, md.k_tile_idx, md.n_batch_idx, md.n_tile_idx)
        if key not in memo:
            memo[key] = producer(nc, md)
        return memo[key]

    return _p


def _slot_graph(ctx, tc, s, Cs, w1_ap, w3_ap, w2_ap, buf_ap, y_ap):
    """One expert slot: gT = silu(w1^T buf), hT = gT * (w3^T buf), y = hT^T-form w2.
    gT/hT stay resident in SBUF; buf and w2 are DMA'd once (memoized producers)."""
    nc = tc.nc
    P = 128
    ISUB = I // P  # 8
    n_tiles = int(math.ceil(Cs / 512.0))
    nt = Cs // n_tiles  # exact n-tile size for mm1: 768->384, 640->320, 512->512
    assert nt * n_tiles == Cs and nt % 64 == 0
    Cp = Cs  # exact tiling: no padded columns needed

    spool = ctx.enter_context(tc.tile_pool(name=f"sb{s}", bufs=1))
    gt_sb = spool.tile([P, ISUB, Cp], mybir.dt.bfloat16, name=f"gt{s}")
    ht_sb = spool.tile([P, ISUB, Cp], mybir.dt.bfloat16, name=f"ht{s}")

    from concourse.kernels.tile_matmul import ShapeInfo

    n_buf_tiles = int(math.ceil(Cs / 512.0)) * (H // 512)  # live buf tiles per slot
    bpool = ctx.enter_context(tc.tile_pool(name=f"bf{s}", bufs=n_buf_tiles + 2))
    buf_producer_raw, buf_shape = dma_from_dram_kxn(bpool, buf_ap)
    buf_producer = _memo_producer(buf_producer_raw)

    w1_pool = ctx.enter_context(tc.tile_pool(name=f"w1p{s}", bufs=12))
    w3_pool = ctx.enter_context(tc.tile_pool(name=f"w3p{s}", bufs=12))
    w1_producer, w1_shape = dma_from_dram_kxm(w1_pool, w1_ap)
    w3_producer, _ = dma_from_dram_kxm(w3_pool, w3_ap)

    def gt_view(md):
        return gt_sb[:, bass.ts(md.m_tile_idx, md.m_subtiles),
                     bass.ds(md.n_tile_idx * md.n_tile, md.n_tile)]

    def ht_view(md):
        return ht_sb[:, bass.ts(md.m_tile_idx, md.m_subtiles),
                     bass.ds(md.n_tile_idx * md.n_tile, md.n_tile)]

    def silu_reduce(nc_, psum, sbuf, md):
        nc_.scalar.activation(sbuf[:], psum[:], mybir.ActivationFunctionType.Silu)

    def mul_reduce(nc_, psum, sbuf, md):
        g = gt_sb[:, bass.ds(md.m_tile_idx * md.m_subtiles + md.m_subtile_idx, 1),
                  bass.ds(md.n_tile_idx * md.n_tile + md.n_subtile_idx * md.n_subtile,
                          md.n_subtile)]
        nc_.vector.tensor_mul(out=sbuf[:], in0=psum[:], in1=g)

    def noop_consumer(nc_, sbuf, md):
        pass

    # mm1a: gT = silu(w1^T @ buf)
    composable_matmul_tile_kernel(
        tc=tc, kxm_shape=w1_shape, kxn_shape=buf_shape, output_type=mybir.dt.bfloat16,
        kxm_producer=w1_producer, kxn_producer=buf_producer,
        mxn_consumer=noop_consumer, mxn_subtile_reducer=silu_reduce,
        mxn_subtile_producer=lambda nc_, md: gt_view(md), psum_n_bufs=2,
        MAX_TILE_SIZE=nt,
    )
    # mm1b: hT = gT * (w3^T @ buf)
    composable_matmul_tile_kernel(
        tc=tc, kxm_shape=w1_shape, kxn_shape=buf_shape, output_type=mybir.dt.bfloat16,
        kxm_producer=w3_producer, kxn_producer=buf_producer,
        mxn_consumer=noop_consumer, mxn_subtile_reducer=mul_reduce,
        mxn_subtile_producer=lambda nc_, md: ht_view(md), psum_n_bufs=2,
        MAX_TILE_SIZE=nt,
    )

    # mm2: y = hT^T-form @ w2 ; kxm tiles are views of ht_sb
    ht_shape = ShapeInfo(pdims=((P, ISUB),), fdims=(Cs,))

    def ht_kxm_producer(nc_, md):
        return ht_sb[:, bass.ds(md.k_tile_idx * md.k_subtiles, md.k_subtiles),
                     bass.ds(md.m_tile_idx * md.m_tile, md.m_tile)]

    w2_pool = ctx.enter_context(tc.tile_pool(name=f"w2p{s}", bufs=9))
    w2_producer_raw, w2_shape = dma_from_dram_kxn(w2_pool, w2_ap)
    w2_producer = _memo_producer(w2_producer_raw)
    composable_matmul_tile_kernel(
        tc=tc, kxm_shape=ht_shape, kxn_shape=w2_shape, output_type=mybir.dt.float32,
        kxm_producer=ht_kxm_producer, kxn_producer=w2_producer,
        mxn_consumer=dma_to_dram_mxn(y_ap),
        mxn_subtile_reducer=scalar_copyback(), psum_n_bufs=1,
    )


def _build_graph(caps, offs):
    from contextlib import ExitStack

    CT = int(offs[-1])
    nc = bacc.Bacc(None, target_bir_lowering=False, debug=False)
    w1_p = nc.declare_dram_parameter("w1s", [SLOTS, H, I], mybir.dt.bfloat16, isOutput=False)
    w3_p = nc.declare_dram_parameter("w3s", [SLOTS, H, I], mybir.dt.bfloat16, isOutput=False)
    w2_p = nc.declare_dram_parameter("w2s", [SLOTS, I, H], mybir.dt.bfloat16, isOutput=False)
    buf_p = nc.declare_dram_parameter("buft", [H, CT], mybir.dt.bfloat16, isOutput=False)
    y_p = nc.declare_dram_parameter("y", [CT, H], mybir.dt.float32, isOutput=True)

    with tile.TileContext(nc) as tc:
        for s in range(SLOTS):
            sl = bass.ds(int(offs[s]), caps[s])
            with ExitStack() as sctx:
                _slot_graph(sctx, tc, s, caps[s], w1_p[s], w3_p[s], w2_p[s],
                            buf_p[:, sl], y_p[sl, :])
    nc.compile()
    return nc


_GRAPH_CACHE = {}


def kernel(x, w_gate, gate_bias, w1, w3, w2):
    x = np.asarray(x, np.float32)
    w_gate = np.asarray(w_gate, np.float32)
    gate_bias = np.asarray(gate_bias, np.float32)

    eidx, w = _route(x, w_gate, gate_bias)
    flat_e, tok, pos, valid, counts = _dispatch_indices(eidx)
    expert_core, expert_slot, caps, offs = _assign_experts(counts)
    CT = int(offs[-1])

    # ---- stage per-core inputs ----
    bufT = np.zeros((M_CORES, H, CT), BF16)
    expert_tokens = {}
    for e in range(E):
        m = (flat_e == e) & valid
        toks = tok[m]  # arrival order == slot positions 0..len-1
        expert_tokens[e] = toks
        c, s = int(expert_core[e]), int(expert_slot[e])
        o = int(offs[s])
        bufT[c][:, o : o + len(toks)] = x[toks].astype(BF16).T

    w1b = np.asarray(w1, np.float32).astype(BF16)
    w3b = np.asarray(w3, np.float32).astype(BF16)
    w2b = np.asarray(w2, np.float32).astype(BF16)
    in_maps = []
    for c in range(M_CORES):
        # experts of core c in slot order
        es = [int(np.where((expert_core == c) & (expert_slot == s))[0][0]) for s in range(SLOTS)]
        in_maps.append({
            "w1s": np.ascontiguousarray(w1b[es]),
            "w3s": np.ascontiguousarray(w3b[es]),
            "w2s": np.ascontiguousarray(w2b[es]),
            "buft": bufT[c],
        })

    # ---- build + run ----
    key = tuple(caps)
    if key not in _GRAPH_CACHE:
        _GRAPH_CACHE[key] = _build_graph(caps, offs)
    nc = _GRAPH_CACHE[key]
    res = run_bass_kernel_spmd(nc, in_maps, core_ids=list(range(M_CORES)))
    ys = np.stack([res.results[c]["y"] for c in range(M_CORES)])  # [8, CT, H] f32

    # ---- combine on host ----
    ycat = ys.reshape(M_CORES * CT, H)
    core_of = expert_core[flat_e]
    slot_of = expert_slot[flat_e]
    col = offs[slot_of] + pos
    row = core_of * CT + col
    row = np.where(valid, row, 0)
    wf = np.where(valid, w.reshape(-1), 0.0).astype(np.float32)
    contrib = ycat[row] * wf[:, None]  # [N*K, H]
    out = contrib.reshape(N, K, H).sum(axis=1)
    return out.astype(np.float32)


# revision 18
# speedup vs baseline: 1.3623x; 1.0344x over previous
"""DeepSeek-style MoE layer on 8 Trainium2 NeuronCores, expert-parallel.

Strategy:
  - Routing (sigmoid gate + group-limited top-k) and token dispatch/combine
    indices are computed on host in fp32 numpy (replicating the reference
    semantics exactly, ~0.1% of total FLOPs).
  - The 32 experts are bin-packed onto 8 cores (4 slots/core) by measured
    load, with per-slot static capacities padded to multiples of 128.
  - Each core runs a Bass/Tile kernel: per slot s,
        gT = silu(w1_s^T @ bufT_s)        [I, Cs]
        hT = gT * (w3_s^T @ bufT_s)       [I, Cs]
        y  = hT^T-form @ w2_s             [Cs, H]
    All matmuls in bf16 with fp32 PSUM accumulation.
  - Combine (gather + weighted sum over the K=8 routes) happens on host.
"""

import math

import ml_dtypes
import numpy as np

import concourse.bass as bass
import concourse.mybir as mybir
import concourse.tile as tile
from concourse import bacc
from concourse.bass_utils import run_bass_kernel_spmd
from concourse.kernels.tile_matmul import (
    composable_matmul_tile_kernel,
    dma_from_dram_kxm,
    dma_from_dram_kxn,
    dma_from_dram_mxn,
    dma_to_dram_mxn,
    k_pool_min_bufs,
    scalar_copyback,
)

# MoE config (matches the reference)
N = 2048
H = 2048
I = 1024
E = 32
K = 8
G = 8
KG = 4
C = 1024
SCALE = 2.5

M_CORES = 8
SLOTS = E // M_CORES  # 4 experts per core

BF16 = ml_dtypes.bfloat16


def _route(x, w_gate, gate_bias):
    """fp32 numpy replication of the reference gate."""
    scores = 1.0 / (1.0 + np.exp(-(x @ w_gate), dtype=np.float32))  # [N, E]
    sb = scores + gate_bias
    grp = sb.reshape(N, G, E // G)
    top2 = -np.sort(-grp, axis=-1)[..., :2]
    gscore = top2.sum(-1)  # [N, G]
    gidx = np.argsort(-gscore, axis=-1, kind="stable")[:, :KG]
    gmask = np.zeros((N, G), bool)
    gmask[np.arange(N)[:, None], gidx] = True
    emask = np.repeat(gmask, E // G, axis=1)
    masked = np.where(emask, sb, -np.inf)
    eidx = np.argsort(-masked, axis=-1, kind="stable")[:, :K]  # [N, K]
    w = np.take_along_axis(scores, eidx, axis=1)
    w = w / w.sum(-1, keepdims=True) * SCALE
    return eidx, w.astype(np.float32)


def _dispatch_indices(eidx):
    """Per-route slot positions, replicating the reference capacity rule."""
    flat_e = eidx.reshape(-1)  # [N*K], token-major arrival order
    tok = np.repeat(np.arange(N), K)
    order = np.argsort(flat_e, kind="stable")
    counts = np.bincount(flat_e, minlength=E)
    starts = np.concatenate([[0], np.cumsum(counts)[:-1]])
    pos_sorted = np.arange(N * K) - np.repeat(starts, counts)
    pos = np.empty(N * K, np.int64)
    pos[order] = pos_sorted
    valid = pos < C
    return flat_e, tok, pos, valid, counts


def _assign_experts(counts):
    """Bin-pack experts: slot s takes load-ranks [8s, 8s+8), snaked across cores."""
    loads = np.minimum(counts, C)
    rank = np.argsort(-loads, kind="stable")
    expert_core = np.empty(E, np.int64)
    expert_slot = np.empty(E, np.int64)
    caps = []
    for s in range(SLOTS):
        grp = rank[M_CORES * s : M_CORES * (s + 1)]
        cores = range(M_CORES) if s % 2 == 0 else range(M_CORES - 1, -1, -1)
        for c, e in zip(cores, grp):
            expert_core[e] = c
            expert_slot[e] = s
        caps.append(int(math.ceil(max(int(loads[e]) for e in grp) / 128.0) * 128))
    offs = np.concatenate([[0], np.cumsum(caps)]).astype(np.int64)
    return expert_core, expert_slot, caps, offs


def _memo_producer(producer):
    memo = {}

    def _p(nc, md):
        key = (md.k_batch_idx, md.k_tile_idx, md.n_batch_idx, md.n_tile_idx)
        if key not in memo:
            memo[key] = producer(nc, md)
        return memo[key]

    return _p


def _slot_graph(ctx, tc, s, Cs, w1_ap, w3_ap, w2_ap, buf_ap, y_ap):
    """One expert slot: gT = silu(w1^T buf), hT = gT * (w3^T buf), y = hT^T-form w2.
    gT/hT stay resident in SBUF; buf and w2 are DMA'd once (memoized producers)."""
    nc = tc.nc
    P = 128
    ISUB = I // P  # 8
    n_tiles = int(math.ceil(Cs / 512.0))
    nt = Cs // n_tiles  # exact n-tile size for mm1: 768->384, 640->320, 512->512
    assert nt * n_tiles == Cs and nt % 64 == 0
    Cp = Cs  # exact tiling: no padded columns needed

    spool = ctx.enter_context(tc.tile_pool(name=f"sb{s}", bufs=1))
    gt_sb = spool.tile([P, ISUB, Cp], mybir.dt.bfloat16, name=f"gt{s}")
    ht_sb = spool.tile([P, ISUB, Cp], mybir.dt.bfloat16, name=f"ht{s}")

    from concourse.kernels.tile_matmul import ShapeInfo

    n_buf_tiles = int(math.ceil(Cs / 512.0)) * (H // 512)  # live buf tiles per slot
    bpool = ctx.enter_context(tc.tile_pool(name=f"bf{s}", bufs=n_buf_tiles + 2))
    buf_producer_raw, buf_shape = dma_from_dram_kxn(bpool, buf_ap)
    buf_producer = _memo_producer(buf_producer_raw)

    w1_pool = ctx.enter_context(tc.tile_pool(name=f"w1p{s}", bufs=12))
    w3_pool = ctx.enter_context(tc.tile_pool(name=f"w3p{s}", bufs=12))
    w1_producer, w1_shape = dma_from_dram_kxm(w1_pool, w1_ap)
    w3_producer, _ = dma_from_dram_kxm(w3_pool, w3_ap)

    def gt_view(md):
        return gt_sb[:, bass.ts(md.m_tile_idx, md.m_subtiles),
                     bass.ds(md.n_tile_idx * md.n_tile, md.n_tile)]

    def ht_view(md):
        return ht_sb[:, bass.ts(md.m_tile_idx, md.m_subtiles),
                     bass.ds(md.n_tile_idx * md.n_tile, md.n_tile)]

    def silu_reduce(nc_, psum, sbuf, md):
        nc_.scalar.activation(sbuf[:], psum[:], mybir.ActivationFunctionType.Silu)

    def mul_reduce(nc_, psum, sbuf, md):
        g = gt_sb[:, bass.ds(md.m_tile_idx * md.m_subtiles + md.m_subtile_idx, 1),
                  bass.ds(md.n_tile_idx * md.n_tile + md.n_subtile_idx * md.n_subtile,
                          md.n_subtile)]
        nc_.vector.tensor_mul(out=sbuf[:], in0=psum[:], in1=g)

    def noop_consumer(nc_, sbuf, md):
        pass

    # mm1a: gT = silu(w1^T @ buf)
    composable_matmul_tile_kernel(
        tc=tc, kxm_shape=w1_shape, kxn_shape=buf_shape, output_type=mybir.dt.bfloat16,
        kxm_producer=w1_producer, kxn_producer=buf_producer,
        mxn_consumer=noop_consumer, mxn_subtile_reducer=silu_reduce,
        mxn_subtile_producer=lambda nc_, md: gt_view(md), psum_n_bufs=2,
        MAX_TILE_SIZE=nt,
    )
    # mm1b: hT = gT * (w3^T @ buf)
    composable_matmul_tile_kernel(
        tc=tc, kxm_shape=w1_shape, kxn_shape=buf_shape, output_type=mybir.dt.bfloat16,
        kxm_producer=w3_producer, kxn_producer=buf_producer,
        mxn_consumer=noop_consumer, mxn_subtile_reducer=mul_reduce,
        mxn_subtile_producer=lambda nc_, md: ht_view(md), psum_n_bufs=2,
        MAX_TILE_SIZE=nt,
    )

    # mm2: y = hT^T-form @ w2 ; kxm tiles are views of ht_sb
    ht_shape = ShapeInfo(pdims=((P, ISUB),), fdims=(Cs,))

    def ht_kxm_producer(nc_, md):
        return ht_sb[:, bass.ds(md.k_tile_idx * md.k_subtiles, md.k_subtiles),
                     bass.ds(md.m_tile_idx * md.m_tile, md.m_tile)]

    w2_pool = ctx.enter_context(tc.tile_pool(name=f"w2p{s}", bufs=9))
    w2_producer_raw, w2_shape = dma_from_dram_kxn(w2_pool, w2_ap)
    w2_producer = _memo_producer(w2_producer_raw)
    composable_matmul_tile_kernel(
        tc=tc, kxm_shape=ht_shape, kxn_shape=w2_shape, output_type=mybir.dt.float32,
        kxm_producer=ht_kxm_producer, kxn_producer=w2_producer,
        mxn_consumer=dma_to_dram_mxn(y_ap),
        mxn_subtile_reducer=scalar_copyback(),
        psum_n_bufs=2 if Cs % 512 else 1,  # <=6 PSUM banks/block on ragged slots
    )


def _build_graph(caps, offs):
    from contextlib import ExitStack

    CT = int(offs[-1])
    nc = bacc.Bacc(None, target_bir_lowering=False, debug=False)
    w1_p = nc.declare_dram_parameter("w1s", [SLOTS, H, I], mybir.dt.bfloat16, isOutput=False)
    w3_p = nc.declare_dram_parameter("w3s", [SLOTS, H, I], mybir.dt.bfloat16, isOutput=False)
    w2_p = nc.declare_dram_parameter("w2s", [SLOTS, I, H], mybir.dt.bfloat16, isOutput=False)
    buf_p = nc.declare_dram_parameter("buft", [H, CT], mybir.dt.bfloat16, isOutput=False)
    y_p = nc.declare_dram_parameter("y", [CT, H], mybir.dt.float32, isOutput=True)

    with tile.TileContext(nc) as tc:
        for s in range(SLOTS):
            sl = bass.ds(int(offs[s]), caps[s])
            with ExitStack() as sctx:
                _slot_graph(sctx, tc, s, caps[s], w1_p[s], w3_p[s], w2_p[s],
                            buf_p[:, sl], y_p[sl, :])
    nc.compile()
    return nc


_GRAPH_CACHE = {}


def kernel(x, w_gate, gate_bias, w1, w3, w2):
    x = np.asarray(x, np.float32)
    w_gate = np.asarray(w_gate, np.float32)
    gate_bias = np.asarray(gate_bias, np.float32)

    eidx, w = _route(x, w_gate, gate_bias)
    flat_e, tok, pos, valid, counts = _dispatch_indices(eidx)
    expert_core, expert_slot, caps, offs = _assign_experts(counts)
    CT = int(offs[-1])

    # ---- stage per-core inputs ----
    bufT = np.zeros((M_CORES, H, CT), BF16)
    expert_tokens = {}
    for e in range(E):
        m = (flat_e == e) & valid
        toks = tok[m]  # arrival order == slot positions 0..len-1
        expert_tokens[e] = toks
        c, s = int(expert_core[e]), int(expert_slot[e])
        o = int(offs[s])
        bufT[c][:, o : o + len(toks)] = x[toks].astype(BF16).T

    w1b = np.asarray(w1, np.float32).astype(BF16)
    w3b = np.asarray(w3, np.float32).astype(BF16)
    w2b = np.asarray(w2, np.float32).astype(BF16)
    in_maps = []
    for c in range(M_CORES):
        # experts of core c in slot order
        es = [int(np.where((expert_core == c) & (expert_slot == s))[0][0]) for s in range(SLOTS)]
        in_maps.append({
            "w1s": np.ascontiguousarray(w1b[es]),
            "w3s": np.ascontiguousarray(w3b[es]),
            "w2s": np.ascontiguousarray(w2b[es]),
            "buft": bufT[c],
        })

    # ---- build + run ----
    key = tuple(caps)
    if key not in _GRAPH_CACHE:
        _GRAPH_CACHE[key] = _build_graph(caps, offs)
    nc = _GRAPH_CACHE[key]
    res = run_bass_kernel_spmd(nc, in_maps, core_ids=list(range(M_CORES)))
    ys = np.stack([res.results[c]["y"] for c in range(M_CORES)])  # [8, CT, H] f32

    # ---- combine on host ----
    ycat = ys.reshape(M_CORES * CT, H)
    core_of = expert_core[flat_e]
    slot_of = expert_slot[flat_e]
    col = offs[slot_of] + pos
    row = core_of * CT + col
    row = np.where(valid, row, 0)
    wf = np.where(valid, w.reshape(-1), 0.0).astype(np.float32)
    contrib = ycat[row] * wf[:, None]  # [N*K, H]
    out = contrib.reshape(N, K, H).sum(axis=1)
    return out.astype(np.float32)
